# revision 17
# baseline (speedup 1.0000x reference)
"""Causal self-attention TRN2 kernel (8 NeuronCores, Megatron-style sharding).

Reference computation (fp32):
    qkv = x @ w_attn.T ; q,k,v split; per-head causal softmax(q k^T/sqrt(hs)) v
    out = y @ w_proj.T
Shapes: x [4, 2048, 1024], w_attn [3072, 1024], w_proj [1024, 1024], 16 heads.

Sharding: core = (b, g) with b = batch 0..3, g = head-group 0..1 (8 heads each).
Each core computes its batch's attention for its 8 heads plus the partial
output projection over its 512 local head-dims; host sums the two partials
per batch (Megatron row-parallel) and transposes back.

Device dataflow is fully transposed ([feature, token] layout) so the PE
contraction dim always sits on partitions with zero on-device transposes:
  qkT[d, t] = waT.T @ xT           (lhsT = waT block, rhs = xT)
  V[t, d]   = xT.T @ waT_v         (lhsT = xT block, rhs = wv)
  S.T[k, q] = KT.T @ QT            (lhsT = KT slice [hs=64, 128], 2 heads share
                                    the PE via row groups 0-1 / 2-3)
  P = exp(S/8) with causal 0/1 mask applied after exp (values are bounded, so
      no max-subtraction is needed); denominators come free from a ones column
      appended to V (y.T matmul has M=65, row 64 = sum_k P)
  y.T[d, q] = V_aug.T @ P          (accumulated over k-tiles in PSUM)
  outT[e, q] = wpT.T @ yT          (partial over local d)
All matmuls run in float32r (full PE rate at N=512, ~1e-4 relative error).
"""

import math

import numpy as np

import concourse.bass as bass
import concourse.tile as tile
from concourse import bacc, mybir
from concourse import bass_utils

F32R = mybir.dt.float32r
F32 = mybir.dt.float32

C = 1024          # embed dim
NH_LOCAL = 8      # heads per core
HS = 64           # head size
DL = NH_LOCAL * HS  # local head-dim total (512)
NCT = C // 128    # c-tiles (contraction tiles) = 8


def build(T: int = 2048):
    """Build + compile the per-core program for sequence length T."""
    NQC = T // 512    # q-chunks
    NKT = T // 128    # k-tiles / t-tiles

    nc = bacc.Bacc(
        "TRN2", target_bir_lowering=False, debug=False, enable_asserts=False
    )

    xT = nc.dram_tensor("xT", [C, T], F32R, kind="ExternalInput").ap()
    waT = nc.dram_tensor("waT", [C, 3 * DL], F32R, kind="ExternalInput").ap()
    wpT = nc.dram_tensor("wpT", [DL, C], F32R, kind="ExternalInput").ap()
    masks = nc.dram_tensor("masks", [4, 128, 512], F32R, kind="ExternalInput").ap()
    outT = nc.dram_tensor("outT", [C, T], F32, kind="ExternalOutput").ap()

    with tile.TileContext(nc) as tc:
        with (
            tc.tile_pool(name="const", bufs=1) as constp,
            tc.tile_pool(name="persist", bufs=1) as persist,
            tc.tile_pool(name="dram", bufs=1, space="DRAM") as dramp,
            # xT (phase A) and QT/KT streams (phase B) share these slots; the
            # 2 extra slots let pair-0's QT/KT prefetch while phase A runs
            tc.tile_pool(name="big", bufs=10) as bigp,
            tc.tile_pool(name="wblk", bufs=12) as wblkp,
            tc.tile_pool(name="wv", bufs=8) as wvp,
            tc.tile_pool(name="wpe", bufs=2) as wpep,
            tc.tile_pool(name="stage", bufs=4) as stagep,
            tc.tile_pool(name="epool", bufs=4) as epool,
            tc.tile_pool(name="misc", bufs=2) as miscp,
            tc.tile_pool(name="ps_small", bufs=2, space="PSUM") as ps_small,
            tc.tile_pool(name="ps_st", bufs=3, space="PSUM") as ps_st,
            tc.tile_pool(name="ps_yt", bufs=3, space="PSUM") as ps_yt,
        ):
            # ---- constants ----
            mask_t = []
            for o in range(4):
                mt = constp.tile([128, 512], F32R, tag=f"mask{o}", name=f"mask{o}")
                nc.sync.dma_start(mt[:], masks[o])
                mask_t.append(mt)

            # ---- persistent activations ----
            va_t = []  # V augmented with ones column: [128, 8*65]
            for tt in range(NKT):
                va = persist.tile(
                    [128, NH_LOCAL * (HS + 1)], F32R, tag=f"va{tt}", name=f"va{tt}"
                )
                va_t.append(va)
            yt_t = []  # y.T per head-pair: [128, T]
            for p in range(4):
                yt = persist.tile([128, T], F32R, tag=f"yt{p}", name=f"yt{p}")
                yt_t.append(yt)

            qk_dram = dramp.tile([2 * DL, T], F32R, tag="qk_dram", name="qk_dram")

            # ================= phase A: projections =================
            xt_t = []
            for ci in range(NCT):
                xt = bigp.tile([128, T], F32R, tag="big", name=f"xt{ci}")
                nc.sync.dma_start(xt[:], xT[128 * ci : 128 * (ci + 1), :])
                xt_t.append(xt)

            # qkT = waT.T @ xT, written to DRAM bounce buffer. Order pairs'
            # q/k d-tiles together (0,4,1,5,...) so pair p's attention inputs
            # are complete early and phase B can prefetch/start sooner.
            for dt in [0, 4, 1, 5, 2, 6, 3, 7]:
                wbs = []
                for ci in range(NCT):
                    wb = wblkp.tile([128, 128], F32R, tag="wblk", name=f"wb{dt}_{ci}")
                    nc.sync.dma_start(
                        wb[:],
                        waT[128 * ci : 128 * (ci + 1), 128 * dt : 128 * (dt + 1)],
                    )
                    wbs.append(wb)
                for jq in range(T // 512):
                    ps = ps_small.tile([128, 512], F32, tag="psA", name="psA")
                    for ci in range(NCT):
                        nc.tensor.matmul(
                            ps[:],
                            wbs[ci][:],
                            xt_t[ci][:, 512 * jq : 512 * (jq + 1)],
                            start=(ci == 0),
                            stop=(ci == NCT - 1),
                        )
                    st = stagep.tile([128, 512], F32R, tag="stage", name="stA")
                    nc.vector.tensor_copy(st[:], ps[:])
                    nc.sync.dma_start(
                        qk_dram[128 * dt : 128 * (dt + 1), 512 * jq : 512 * (jq + 1)],
                        st[:],
                    )

            # V = xT.T @ wv  (+ ones column per head)
            wv_t = []
            for ci in range(NCT):
                wv = wvp.tile([128, DL], F32R, tag="wv", name=f"wv{ci}")
                nc.sync.dma_start(wv[:], waT[128 * ci : 128 * (ci + 1), 2 * DL :])
                wv_t.append(wv)
            for tt in range(NKT):
                ps = ps_small.tile([128, 512], F32, tag="psA", name="psV")
                for ci in range(NCT):
                    nc.tensor.matmul(
                        ps[:],
                        xt_t[ci][:, 128 * tt : 128 * (tt + 1)],
                        wv_t[ci][:],
                        start=(ci == 0),
                        stop=(ci == NCT - 1),
                    )
                va = va_t[tt]
                va3 = va[:].rearrange("p (h d) -> p h d", d=HS + 1)
                ps3 = ps[:].rearrange("p (h d) -> p h d", d=HS)
                nc.vector.tensor_copy(va3[:, :, 0:HS], ps3[:])
                nc.vector.memset(va3[:, :, HS].bitcast(F32), 1.0)

            # ================= phase B: attention =================
            for p in range(4):  # head pairs
                qt = bigp.tile([128, T], F32R, tag="big", name=f"qt{p}")
                nc.sync.dma_start(qt[:], qk_dram[128 * p : 128 * (p + 1), :])
                kt = bigp.tile([128, T], F32R, tag="big", name=f"kt{p}")
                nc.sync.dma_start(kt[:], qk_dram[DL + 128 * p : DL + 128 * (p + 1), :])

                for j in range(NQC):
                    qs = slice(512 * j, 512 * (j + 1))
                    ytps = [
                        ps_yt.tile([HS + 1, 512], F32, tag="ytp", name="ytp0"),
                        ps_yt.tile([HS + 1, 512], F32, tag="ytp", name="ytp1"),
                    ]
                    n_kt = 4 * j + 4
                    # Software pipeline: S.T/exp run one k-tile ahead of the
                    # consuming y.T matmuls so the PE never waits on ACT.
                    ets = {}
                    for i in range(n_kt + 1):
                        if i < n_kt:
                            ks = slice(128 * i, 128 * (i + 1))
                            for h in range(2):  # head in pair, row-group packed
                                hp = slice(64 * h, 64 * (h + 1))
                                st = ps_st.tile(
                                    [128, 512], F32, tag="stp", name="stp"
                                )
                                nc.tensor.matmul(
                                    st[:], kt[hp, ks], qt[hp, qs],
                                    start=True, stop=True,
                                )
                                et = epool.tile([128, 512], F32R, tag="et", name="et")
                                nc.scalar.activation(
                                    et[:],
                                    st[:],
                                    mybir.ActivationFunctionType.Exp,
                                    scale=1.0 / math.sqrt(HS),
                                )
                                if i >= 4 * j:  # diagonal tile: causal mask
                                    nc.vector.tensor_mul(
                                        et[:], et[:], mask_t[i - 4 * j][:]
                                    )
                                ets[(i, h)] = et
                        ic = i - 1  # consume previous k-tile
                        if ic >= 0:
                            for h in range(2):
                                hh = 2 * p + h
                                nc.tensor.matmul(
                                    ytps[h][:],
                                    va_t[ic][:, 65 * hh : 65 * hh + 65],
                                    ets.pop((ic, h)),
                                    start=(ic == 0),
                                    stop=(ic == n_kt - 1),
                                )
                    for h in range(2):
                        # Drain PSUM immediately (one cheap copy) so the bank
                        # frees for the next chunk; normalize from SBUF off
                        # the PE critical path: yT = y * broadcast(1/sum).
                        yu = miscp.tile([HS + 1, 512], F32, tag="yu", name="yu")
                        nc.vector.tensor_copy(yu[:], ytps[h][:])
                        rc = miscp.tile([1, 512], F32, tag="rc", name="rc")
                        nc.vector.reciprocal(rc[:], yu[HS : HS + 1, :])
                        rb = miscp.tile([64, 512], F32, tag="rb", name="rb")
                        nc.gpsimd.partition_broadcast(rb[:], rc[:])
                        nc.vector.tensor_mul(
                            yt_t[p][64 * h : 64 * (h + 1), qs],
                            yu[0:HS, :],
                            rb[:],
                        )

            # ================= phase C: output projection =================
            for e in range(C // 128):
                wps = []
                for p in range(4):
                    wp = wpep.tile([128, 128], F32R, tag=f"wpe{p}", name=f"wpe{p}")
                    nc.sync.dma_start(
                        wp[:],
                        wpT[128 * p : 128 * (p + 1), 128 * e : 128 * (e + 1)],
                    )
                    wps.append(wp)
                for jq in range(T // 512):
                    ps = ps_small.tile([128, 512], F32, tag="psA", name="psC")
                    for p in range(4):
                        nc.tensor.matmul(
                            ps[:],
                            wps[p][:],
                            yt_t[p][:, 512 * jq : 512 * (jq + 1)],
                            start=(p == 0),
                            stop=(p == 3),
                        )
                    ot = stagep.tile([128, 512], F32, tag="stage", name="stC")
                    nc.scalar.copy(ot[:], ps[:])
                    nc.sync.dma_start(
                        outT[128 * e : 128 * (e + 1), 512 * jq : 512 * (jq + 1)],
                        ot[:],
                    )

    nc.compile()
    return nc


_CACHE: dict = {}
_LAST_IN_MAPS = None


def _get_nc(T: int):
    if T not in _CACHE:
        _CACHE[T] = build(T)
    return _CACHE[T]


def _make_masks() -> np.ndarray:
    kk = np.arange(128)[:, None]
    qq = np.arange(512)[None, :]
    return np.stack(
        [(qq >= 128 * o + kk).astype(np.float32) for o in range(4)]
    )


def kernel(x: np.ndarray, w_attn: np.ndarray, w_proj: np.ndarray) -> np.ndarray:
    B, T, C_ = x.shape
    nc = _get_nc(T)
    masks = _make_masks()

    in_maps = []
    for core in range(8):
        b, g = core // 2, core % 2
        heads = range(8 * g, 8 * g + 8)
        rows = []
        for base in (0, C_, 2 * C_):  # q, k, v sections of w_attn
            for H in heads:
                rows.extend(range(base + 64 * H, base + 64 * H + 64))
        waT_l = np.ascontiguousarray(np.asarray(w_attn)[rows, :].T.astype(np.float32))
        dcols = [c for H in heads for c in range(64 * H, 64 * H + 64)]
        wpT_l = np.ascontiguousarray(np.asarray(w_proj)[:, dcols].T.astype(np.float32))
        xT_l = np.ascontiguousarray(np.asarray(x[b]).T.astype(np.float32))
        in_maps.append({"xT": xT_l, "waT": waT_l, "wpT": wpT_l, "masks": masks})

    global _LAST_IN_MAPS
    _LAST_IN_MAPS = in_maps
    res = bass_utils.run_bass_kernel_spmd(nc, in_maps, core_ids=list(range(8)))
    out = np.empty((B, T, C_), dtype=np.float32)
    for b in range(B):
        out[b] = (
            res.results[2 * b]["outT"].astype(np.float32)
            + res.results[2 * b + 1]["outT"].astype(np.float32)
        ).T
    return out


# revision 18
# speedup vs baseline: 1.2151x; 1.2151x over previous
"""Causal self-attention TRN2 kernel (8 NeuronCores, Megatron-style sharding).

Reference computation (fp32):
    qkv = x @ w_attn.T ; q,k,v split; per-head causal softmax(q k^T/sqrt(hs)) v
    out = y @ w_proj.T
Shapes: x [4, 2048, 1024], w_attn [3072, 1024], w_proj [1024, 1024], 16 heads.

Sharding: core = (b, g) with b = batch 0..3, g = head-group 0..1 (8 heads each).
Each core computes its batch's attention for its 8 heads plus the partial
output projection over its 512 local head-dims; host sums the two partials
per batch (Megatron row-parallel) and transposes back.

Device dataflow is fully transposed ([feature, token] layout) so the PE
contraction dim always sits on partitions with zero on-device transposes:
  qkT[d, t] = waT.T @ xT           (lhsT = waT block, rhs = xT)
  V[t, d]   = xT.T @ waT_v         (lhsT = xT block, rhs = wv)
  S.T[k, q] = KT.T @ QT            (lhsT = KT slice [hs=64, 128], 2 heads share
                                    the PE via row groups 0-1 / 2-3)
  P = exp(S/8) with causal 0/1 mask applied after exp (values are bounded, so
      no max-subtraction is needed); denominators come free from a ones column
      appended to V (y.T matmul has M=65, row 64 = sum_k P)
  y.T[d, q] = V_aug.T @ P          (accumulated over k-tiles in PSUM)
  outT[e, q] = wpT.T @ yT          (partial over local d)
All matmuls run in float32r (full PE rate at N=512, ~1e-4 relative error).
"""

import math

import numpy as np

import concourse.bass as bass
import concourse.tile as tile
from concourse import bacc, mybir
from concourse import bass_utils

F32R = mybir.dt.float32r
F32 = mybir.dt.float32
BF16 = mybir.dt.bfloat16

# attention-phase matmul dtype: BF16 runs the PE at 2x the float32r rate at
# ~10x the rounding error; projections always stay float32r
ATT_DT = BF16

C = 1024          # embed dim
NH_LOCAL = 8      # heads per core
HS = 64           # head size
DL = NH_LOCAL * HS  # local head-dim total (512)
NCT = C // 128    # c-tiles (contraction tiles) = 8


def build(T: int = 2048):
    """Build + compile the per-core program for sequence length T."""
    NQC = T // 512    # q-chunks
    NKT = T // 128    # k-tiles / t-tiles

    nc = bacc.Bacc(
        "TRN2", target_bir_lowering=False, debug=False, enable_asserts=False
    )

    xT = nc.dram_tensor("xT", [C, T], F32R, kind="ExternalInput").ap()
    waT = nc.dram_tensor("waT", [C, 3 * DL], F32R, kind="ExternalInput").ap()
    wpT = nc.dram_tensor("wpT", [DL, C], F32R, kind="ExternalInput").ap()
    masks = nc.dram_tensor("masks", [4, 128, 512], ATT_DT, kind="ExternalInput").ap()
    outT = nc.dram_tensor("outT", [C, T], F32, kind="ExternalOutput").ap()

    with tile.TileContext(nc) as tc:
        with (
            tc.tile_pool(name="const", bufs=1) as constp,
            tc.tile_pool(name="persist", bufs=1) as persist,
            tc.tile_pool(name="dram", bufs=1, space="DRAM") as dramp,
            # xT (phase A) and QT/KT streams (phase B) share these slots; the
            # 2 extra slots let pair-0's QT/KT prefetch while phase A runs
            tc.tile_pool(name="big", bufs=10) as bigp,
            tc.tile_pool(name="wblk", bufs=12) as wblkp,
            tc.tile_pool(name="wv", bufs=8) as wvp,
            tc.tile_pool(name="wpe", bufs=2) as wpep,
            tc.tile_pool(name="stage", bufs=4) as stagep,
            tc.tile_pool(name="epool", bufs=4) as epool,
            tc.tile_pool(name="misc", bufs=2) as miscp,
            tc.tile_pool(name="ps_small", bufs=2, space="PSUM") as ps_small,
            tc.tile_pool(name="ps_st", bufs=3, space="PSUM") as ps_st,
            tc.tile_pool(name="ps_yt", bufs=3, space="PSUM") as ps_yt,
        ):
            # ---- constants ----
            mask_t = []
            for o in range(4):
                mt = constp.tile([128, 512], ATT_DT, tag=f"mask{o}", name=f"mask{o}")
                nc.sync.dma_start(mt[:], masks[o])
                mask_t.append(mt)

            # ---- persistent activations ----
            va_t = []  # V augmented with ones column: [128, 8*65]
            for tt in range(NKT):
                va = persist.tile(
                    [128, NH_LOCAL * (HS + 1)], ATT_DT, tag=f"va{tt}", name=f"va{tt}"
                )
                va_t.append(va)
            yt_t = []  # y.T per head-pair: [128, T]
            for p in range(4):
                yt = persist.tile([128, T], F32R, tag=f"yt{p}", name=f"yt{p}")
                yt_t.append(yt)

            qk_dram = dramp.tile([2 * DL, T], ATT_DT, tag="qk_dram", name="qk_dram")

            # ================= phase A: projections =================
            xt_t = []
            for ci in range(NCT):
                xt = bigp.tile([128, T], F32R, tag="big", name=f"xt{ci}")
                nc.sync.dma_start(xt[:], xT[128 * ci : 128 * (ci + 1), :])
                xt_t.append(xt)

            # qkT = waT.T @ xT, written to DRAM bounce buffer. Order pairs'
            # q/k d-tiles together (0,4,1,5,...) so pair p's attention inputs
            # are complete early and phase B can prefetch/start sooner.
            for dt in [0, 4, 1, 5, 2, 6, 3, 7]:
                wbs = []
                for ci in range(NCT):
                    wb = wblkp.tile([128, 128], F32R, tag="wblk", name=f"wb{dt}_{ci}")
                    nc.sync.dma_start(
                        wb[:],
                        waT[128 * ci : 128 * (ci + 1), 128 * dt : 128 * (dt + 1)],
                    )
                    wbs.append(wb)
                for jq in range(T // 512):
                    ps = ps_small.tile([128, 512], F32, tag="psA", name="psA")
                    for ci in range(NCT):
                        nc.tensor.matmul(
                            ps[:],
                            wbs[ci][:],
                            xt_t[ci][:, 512 * jq : 512 * (jq + 1)],
                            start=(ci == 0),
                            stop=(ci == NCT - 1),
                        )
                    st = stagep.tile([128, 512], ATT_DT, tag="stage", name="stA")
                    nc.vector.tensor_copy(st[:], ps[:])
                    nc.sync.dma_start(
                        qk_dram[128 * dt : 128 * (dt + 1), 512 * jq : 512 * (jq + 1)],
                        st[:],
                    )

            # V = xT.T @ wv  (+ ones column per head)
            wv_t = []
            for ci in range(NCT):
                wv = wvp.tile([128, DL], F32R, tag="wv", name=f"wv{ci}")
                nc.sync.dma_start(wv[:], waT[128 * ci : 128 * (ci + 1), 2 * DL :])
                wv_t.append(wv)
            for tt in range(NKT):
                ps = ps_small.tile([128, 512], F32, tag="psA", name="psV")
                for ci in range(NCT):
                    nc.tensor.matmul(
                        ps[:],
                        xt_t[ci][:, 128 * tt : 128 * (tt + 1)],
                        wv_t[ci][:],
                        start=(ci == 0),
                        stop=(ci == NCT - 1),
                    )
                va = va_t[tt]
                va3 = va[:].rearrange("p (h d) -> p h d", d=HS + 1)
                ps3 = ps[:].rearrange("p (h d) -> p h d", d=HS)
                nc.vector.tensor_copy(va3[:, :, 0:HS], ps3[:])
                if ATT_DT == BF16:
                    nc.vector.memset(va3[:, :, HS].bitcast(mybir.dt.uint16), 0x3F80)
                else:
                    nc.vector.memset(va3[:, :, HS].bitcast(F32), 1.0)

            # ================= phase B: attention =================
            for p in range(4):  # head pairs
                qt = bigp.tile([128, T], ATT_DT, tag="big", name=f"qt{p}")
                nc.sync.dma_start(qt[:], qk_dram[128 * p : 128 * (p + 1), :])
                kt = bigp.tile([128, T], ATT_DT, tag="big", name=f"kt{p}")
                nc.sync.dma_start(kt[:], qk_dram[DL + 128 * p : DL + 128 * (p + 1), :])

                for j in range(NQC):
                    qs = slice(512 * j, 512 * (j + 1))
                    ytps = [
                        ps_yt.tile([HS + 1, 512], F32, tag="ytp", name="ytp0"),
                        ps_yt.tile([HS + 1, 512], F32, tag="ytp", name="ytp1"),
                    ]
                    n_kt = 4 * j + 4
                    # Software pipeline: S.T/exp run one k-tile ahead of the
                    # consuming y.T matmuls so the PE never waits on ACT.
                    ets = {}
                    for i in range(n_kt + 1):
                        if i < n_kt:
                            ks = slice(128 * i, 128 * (i + 1))
                            for h in range(2):  # head in pair, row-group packed
                                hp = slice(64 * h, 64 * (h + 1))
                                st = ps_st.tile(
                                    [128, 512], F32, tag="stp", name="stp"
                                )
                                nc.tensor.matmul(
                                    st[:], kt[hp, ks], qt[hp, qs],
                                    start=True, stop=True,
                                )
                                et = epool.tile([128, 512], ATT_DT, tag="et", name="et")
                                nc.scalar.activation(
                                    et[:],
                                    st[:],
                                    mybir.ActivationFunctionType.Exp,
                                    scale=1.0 / math.sqrt(HS),
                                )
                                if i >= 4 * j:  # diagonal tile: causal mask
                                    nc.vector.tensor_mul(
                                        et[:], et[:], mask_t[i - 4 * j][:]
                                    )
                                ets[(i, h)] = et
                        ic = i - 1  # consume previous k-tile
                        if ic >= 0:
                            for h in range(2):
                                hh = 2 * p + h
                                nc.tensor.matmul(
                                    ytps[h][:],
                                    va_t[ic][:, 65 * hh : 65 * hh + 65],
                                    ets.pop((ic, h)),
                                    start=(ic == 0),
                                    stop=(ic == n_kt - 1),
                                )
                    for h in range(2):
                        # Drain PSUM immediately (one cheap copy) so the bank
                        # frees for the next chunk; normalize from SBUF off
                        # the PE critical path: yT = y * broadcast(1/sum).
                        yu = miscp.tile([HS + 1, 512], F32, tag="yu", name="yu")
                        nc.vector.tensor_copy(yu[:], ytps[h][:])
                        rc = miscp.tile([1, 512], F32, tag="rc", name="rc")
                        nc.vector.reciprocal(rc[:], yu[HS : HS + 1, :])
                        rb = miscp.tile([64, 512], F32, tag="rb", name="rb")
                        nc.gpsimd.partition_broadcast(rb[:], rc[:])
                        nc.vector.tensor_mul(
                            yt_t[p][64 * h : 64 * (h + 1), qs],
                            yu[0:HS, :],
                            rb[:],
                        )

            # ================= phase C: output projection =================
            for e in range(C // 128):
                wps = []
                for p in range(4):
                    wp = wpep.tile([128, 128], F32R, tag=f"wpe{p}", name=f"wpe{p}")
                    nc.sync.dma_start(
                        wp[:],
                        wpT[128 * p : 128 * (p + 1), 128 * e : 128 * (e + 1)],
                    )
                    wps.append(wp)
                for jq in range(T // 512):
                    ps = ps_small.tile([128, 512], F32, tag="psA", name="psC")
                    for p in range(4):
                        nc.tensor.matmul(
                            ps[:],
                            wps[p][:],
                            yt_t[p][:, 512 * jq : 512 * (jq + 1)],
                            start=(p == 0),
                            stop=(p == 3),
                        )
                    ot = stagep.tile([128, 512], F32, tag="stage", name="stC")
                    nc.scalar.copy(ot[:], ps[:])
                    nc.sync.dma_start(
                        outT[128 * e : 128 * (e + 1), 512 * jq : 512 * (jq + 1)],
                        ot[:],
                    )

    nc.compile()
    return nc


_CACHE: dict = {}
_LAST_IN_MAPS = None


def _get_nc(T: int):
    if T not in _CACHE:
        _CACHE[T] = build(T)
    return _CACHE[T]


def _make_masks() -> np.ndarray:
    kk = np.arange(128)[:, None]
    qq = np.arange(512)[None, :]
    import ml_dtypes

    dt = ml_dtypes.bfloat16 if ATT_DT == BF16 else np.float32
    return np.stack([(qq >= 128 * o + kk).astype(dt) for o in range(4)])


def kernel(x: np.ndarray, w_attn: np.ndarray, w_proj: np.ndarray) -> np.ndarray:
    B, T, C_ = x.shape
    nc = _get_nc(T)
    masks = _make_masks()

    in_maps = []
    for core in range(8):
        b, g = core // 2, core % 2
        heads = range(8 * g, 8 * g + 8)
        rows = []
        for base in (0, C_, 2 * C_):  # q, k, v sections of w_attn
            for H in heads:
                rows.extend(range(base + 64 * H, base + 64 * H + 64))
        waT_l = np.ascontiguousarray(np.asarray(w_attn)[rows, :].T.astype(np.float32))
        dcols = [c for H in heads for c in range(64 * H, 64 * H + 64)]
        wpT_l = np.ascontiguousarray(np.asarray(w_proj)[:, dcols].T.astype(np.float32))
        xT_l = np.ascontiguousarray(np.asarray(x[b]).T.astype(np.float32))
        in_maps.append({"xT": xT_l, "waT": waT_l, "wpT": wpT_l, "masks": masks})

    global _LAST_IN_MAPS
    _LAST_IN_MAPS = in_maps
    res = bass_utils.run_bass_kernel_spmd(nc, in_maps, core_ids=list(range(8)))
    out = np.empty((B, T, C_), dtype=np.float32)
    for b in range(B):
        out[b] = (
            res.results[2 * b]["outT"].astype(np.float32)
            + res.results[2 * b + 1]["outT"].astype(np.float32)
        ).T
    return out


# revision 23
# speedup vs baseline: 1.4722x; 1.2116x over previous
"""Causal self-attention TRN2 kernel (8 NeuronCores, Megatron-style sharding).

Reference computation (fp32):
    qkv = x @ w_attn.T ; q,k,v split; per-head causal softmax(q k^T/sqrt(hs)) v
    out = y @ w_proj.T
Shapes: x [4, 2048, 1024], w_attn [3072, 1024], w_proj [1024, 1024], 16 heads.

Sharding: core = (b, g) with b = batch 0..3, g = head-group 0..1 (8 heads each).
Each core computes its batch's attention for its 8 heads plus the partial
output projection over its 512 local head-dims; host sums the two partials
per batch (Megatron row-parallel) and transposes back.

Device dataflow is fully transposed ([feature, token] layout) so the PE
contraction dim always sits on partitions with zero on-device transposes:
  qkT[d, t] = waT.T @ xT           (lhsT = waT block, rhs = xT)
  V[t, d]   = xT.T @ waT_v         (lhsT = xT block, rhs = wv)
  S.T[k, q] = KT.T @ QT            (lhsT = KT slice [hs=64, 128]; the two heads
                                    of a pair use PE row groups 0-1 / 2-3 and
                                    write the two halves of a [128,1024] PSUM)
  P = exp(S/8), computed only on causal columns (values are bounded, so no
      max-subtraction is needed); a [128,128] lower-tri mask fixes the
      diagonal block, zero-fill covers the below-diagonal prefix; softmax
      denominators come free from a ones column appended to V (the y.T
      matmul has M=65, row 64 = sum_k P)
  y.T[d, q] = V_aug.T @ P          (accumulated over k-tiles in PSUM)
  outT[e, q] = wpT.T @ yT          (partial over local d)
Matmuls run in bf16 (fp32 PSUM accumulation); softmax sums/normalization in
fp32.
"""

import math

import numpy as np

import concourse.bass as bass
import concourse.tile as tile
from concourse import bacc, mybir
from concourse import bass_utils

F32 = mybir.dt.float32
BF16 = mybir.dt.bfloat16
DT = BF16

C = 1024          # embed dim
NH_LOCAL = 8      # heads per core
HS = 64           # head size
DL = NH_LOCAL * HS  # local head-dim total (512)
NCT = C // 128    # c-tiles (contraction tiles) = 8


def build(T: int = 2048):
    """Build + compile the per-core program for sequence length T."""
    NQC = T // 512    # q-chunks
    NKT = T // 128    # k-tiles / t-tiles

    nc = bacc.Bacc(
        "TRN2", target_bir_lowering=False, debug=False, enable_asserts=False
    )

    xT = nc.dram_tensor("xT", [C, T], DT, kind="ExternalInput").ap()
    waT = nc.dram_tensor("waT", [C, 3 * DL], DT, kind="ExternalInput").ap()
    wpT = nc.dram_tensor("wpT", [DL, C], DT, kind="ExternalInput").ap()
    tri = nc.dram_tensor("tri", [128, 128], DT, kind="ExternalInput").ap()
    outT = nc.dram_tensor("outT", [C, T], F32, kind="ExternalOutput").ap()

    with tile.TileContext(nc) as tc:
        with (
            tc.tile_pool(name="const", bufs=1) as constp,
            tc.tile_pool(name="persist", bufs=1) as persist,
            tc.tile_pool(name="xtp", bufs=8) as xtp,
            tc.tile_pool(name="wblk", bufs=16) as wblkp,
            tc.tile_pool(name="wv", bufs=8) as wvp,
            tc.tile_pool(name="wpe", bufs=8) as wpep,
            tc.tile_pool(name="stage", bufs=4) as stagep,
            tc.tile_pool(name="epool", bufs=4) as epool,
            tc.tile_pool(name="misc", bufs=3) as miscp,
            tc.tile_pool(name="yup", bufs=10) as yup,
            tc.tile_pool(name="ps_small", bufs=2, space="PSUM") as ps_small,
            tc.tile_pool(name="ps_st", bufs=2, space="PSUM") as ps_st,
            tc.tile_pool(name="ps_yt", bufs=2, space="PSUM") as ps_yt,
        ):
            # ---- constants ----
            tri_t = constp.tile([128, 128], DT, tag="tri", name="tri_t")
            nc.sync.dma_start(tri_t[:], tri[:])

            # ---- persistent activations ----
            va_t = []  # V augmented with ones column: [128, 8*65]
            for tt in range(NKT):
                va = persist.tile(
                    [128, NH_LOCAL * (HS + 1)], DT, tag=f"va{tt}", name=f"va{tt}"
                )
                va_t.append(va)
            yt_t = []  # y.T per head-pair: [128, T]
            for p in range(4):
                yt = persist.tile([128, T], DT, tag=f"yt{p}", name=f"yt{p}")
                yt_t.append(yt)
            qk_t = []  # qkT resident: tiles 0-3 = QT pairs, 4-7 = KT pairs
            for dt in range(8):
                qk = persist.tile([128, T], DT, tag=f"qk{dt}", name=f"qk{dt}")
                qk_t.append(qk)

            # ================= phase A: projections =================
            xt_t = []
            for ci in range(NCT):
                xt = xtp.tile([128, T], DT, tag="xt", name=f"xt{ci}")
                nc.sync.dma_start(xt[:], xT[128 * ci : 128 * (ci + 1), :])
                xt_t.append(xt)

            # qkT = waT.T @ xT (resident in SBUF)
            for dt in range(8):
                wbs = []
                for ci in range(NCT):
                    wb = wblkp.tile([128, 128], DT, tag="wblk", name=f"wb{dt}_{ci}")
                    nc.sync.dma_start(
                        wb[:],
                        waT[128 * ci : 128 * (ci + 1), 128 * dt : 128 * (dt + 1)],
                    )
                    wbs.append(wb)
                for jq in range(T // 512):
                    ps = ps_small.tile([128, 512], F32, tag="psA", name="psA")
                    for ci in range(NCT):
                        nc.tensor.matmul(
                            ps[:],
                            wbs[ci][:],
                            xt_t[ci][:, 512 * jq : 512 * (jq + 1)],
                            start=(ci == 0),
                            stop=(ci == NCT - 1),
                        )
                    nc.vector.tensor_copy(
                        qk_t[dt][:, 512 * jq : 512 * (jq + 1)], ps[:]
                    )

            # V = xT.T @ wv  (+ ones column per head)
            wv_t = []
            for ci in range(NCT):
                wv = wvp.tile([128, DL], DT, tag="wv", name=f"wv{ci}")
                nc.sync.dma_start(wv[:], waT[128 * ci : 128 * (ci + 1), 2 * DL :])
                wv_t.append(wv)
            for tt in range(NKT):
                ps = ps_small.tile([128, 512], F32, tag="psA", name="psV")
                for ci in range(NCT):
                    nc.tensor.matmul(
                        ps[:],
                        xt_t[ci][:, 128 * tt : 128 * (tt + 1)],
                        wv_t[ci][:],
                        start=(ci == 0),
                        stop=(ci == NCT - 1),
                    )
                va = va_t[tt]
                va3 = va[:].rearrange("p (h d) -> p h d", d=HS + 1)
                ps3 = ps[:].rearrange("p (h d) -> p h d", d=HS)
                nc.vector.tensor_copy(va3[:, :, 0:HS], ps3[:])
                nc.vector.memset(va3[:, :, HS].bitcast(mybir.dt.uint16), 0x3F80)

            # ================= phase B: attention =================
            EXPF = mybir.ActivationFunctionType.Exp
            ISCALE = 1.0 / math.sqrt(HS)
            for p in range(4):  # head pairs
                qt, kt = qk_t[p], qk_t[4 + p]
                # (j, h) softmax sums parked at 32-aligned partitions so one
                # [128,512] reciprocal covers 4 chunks at once
                NS = (2 * NQC + 3) // 4
                sums = [
                    miscp.tile([128, 512], F32, tag="sums", name=f"sums{s}")
                    for s in range(NS)
                ]
                rcs = [
                    miscp.tile([128, 512], F32, tag="rcs", name=f"rcs{s}")
                    for s in range(NS)
                ]
                for s in range(NS):
                    nc.vector.memset(sums[s][:], 1.0)
                yus = {}
                for j in range(NQC):
                    qs = slice(512 * j, 512 * (j + 1))
                    ytps = [
                        ps_yt.tile([HS + 1, 512], F32, tag="ytp", name="ytp0"),
                        ps_yt.tile([HS + 1, 512], F32, tag="ytp", name="ytp1"),
                    ]
                    n_kt = 4 * j + 4
                    # Software pipeline: S.T/exp run one k-tile ahead of the
                    # consuming y.T matmuls so the PE never waits on ACT.
                    ets = {}
                    for i in range(n_kt + 1):
                        if i < n_kt:
                            ks = slice(128 * i, 128 * (i + 1))
                            # both heads' S.T into one [128,1024] PSUM
                            st = ps_st.tile([128, 1024], F32, tag="stp", name="stp")
                            for h in range(2):
                                hp = slice(64 * h, 64 * (h + 1))
                                nc.tensor.matmul(
                                    st[:, 512 * h : 512 * (h + 1)],
                                    kt[hp, ks],
                                    qt[hp, qs],
                                    start=True,
                                    stop=True,
                                )
                            et = epool.tile([128, 1024], DT, tag="et", name="et")
                            o = 128 * (i - 4 * j)  # diag block offset, <0 if past
                            if o <= 0:  # fully causal k-tile
                                nc.scalar.activation(
                                    et[:], st[:], EXPF, scale=ISCALE
                                )
                            else:  # diagonal-crossing: skip invalid prefix
                                for h in range(2):
                                    c0 = 512 * h
                                    nc.vector.memset(
                                        et[:, c0 : c0 + o].bitcast(
                                            mybir.dt.uint16
                                        ),
                                        0,
                                    )
                                    nc.scalar.activation(
                                        et[:, c0 + o : c0 + 512],
                                        st[:, c0 + o : c0 + 512],
                                        EXPF,
                                        scale=ISCALE,
                                    )
                            if o >= 0:  # mask the diagonal 128-block per head
                                for h in range(2):
                                    db = slice(512 * h + o, 512 * h + o + 128)
                                    nc.vector.tensor_mul(
                                        et[:, db], et[:, db], tri_t[:]
                                    )
                            ets[i] = et
                        ic = i - 1  # consume previous k-tile
                        if ic >= 0:
                            et = ets.pop(ic)
                            for h in range(2):
                                hh = 2 * p + h
                                nc.tensor.matmul(
                                    ytps[h][:],
                                    va_t[ic][:, 65 * hh : 65 * hh + 65],
                                    et[:, 512 * h : 512 * (h + 1)],
                                    start=(ic == 0),
                                    stop=(ic == n_kt - 1),
                                )
                    for h in range(2):
                        # drain PSUM fast; park the sum row 32-aligned
                        yu = yup.tile([HS + 1, 512], F32, tag="yu", name="yu")
                        nc.vector.tensor_copy(yu[:], ytps[h][:])
                        yus[(j, h)] = yu
                        r = 2 * j + h
                        nc.vector.tensor_copy(
                            sums[r // 4][32 * (r % 4) : 32 * (r % 4) + 1, :],
                            yu[HS : HS + 1, :],
                        )
                # batched reciprocals (2 ops per pair), then normalize
                for s in range(NS):
                    nc.vector.reciprocal(rcs[s][:], sums[s][:])
                for j in range(NQC):
                    qs = slice(512 * j, 512 * (j + 1))
                    for h in range(2):
                        r = 2 * j + h
                        # HW partition_broadcast reads partition 0 only, so
                        # stage the reciprocal row there first
                        r0 = miscp.tile([1, 512], F32, tag="r0", name="r0")
                        nc.vector.tensor_copy(
                            r0[:], rcs[r // 4][32 * (r % 4) : 32 * (r % 4) + 1, :]
                        )
                        rb = miscp.tile([64, 512], F32, tag="rb", name="rb")
                        nc.gpsimd.partition_broadcast(rb[:], r0[:])
                        nc.vector.tensor_mul(
                            yt_t[p][64 * h : 64 * (h + 1), qs],
                            yus.pop((j, h))[0:HS, :],
                            rb[:],
                        )

            # ================= phase C: output projection =================
            for e in range(C // 128):
                wps = []
                for p in range(4):
                    wp = wpep.tile([128, 128], DT, tag="wpe", name=f"wpe{p}")
                    nc.sync.dma_start(
                        wp[:],
                        wpT[128 * p : 128 * (p + 1), 128 * e : 128 * (e + 1)],
                    )
                    wps.append(wp)
                for jq in range(T // 512):
                    ps = ps_small.tile([128, 512], F32, tag="psA", name="psC")
                    for p in range(4):
                        nc.tensor.matmul(
                            ps[:],
                            wps[p][:],
                            yt_t[p][:, 512 * jq : 512 * (jq + 1)],
                            start=(p == 0),
                            stop=(p == 3),
                        )
                    ot = stagep.tile([128, 512], F32, tag="stage", name="stC")
                    nc.scalar.copy(ot[:], ps[:])
                    nc.sync.dma_start(
                        outT[128 * e : 128 * (e + 1), 512 * jq : 512 * (jq + 1)],
                        ot[:],
                    )

    nc.compile()
    return nc


_CACHE: dict = {}
_LAST_IN_MAPS = None


def _get_nc(T: int):
    if T not in _CACHE:
        _CACHE[T] = build(T)
    return _CACHE[T]


def kernel(x: np.ndarray, w_attn: np.ndarray, w_proj: np.ndarray) -> np.ndarray:
    import ml_dtypes

    B, T, C_ = x.shape
    nc = _get_nc(T)
    bf = ml_dtypes.bfloat16
    kk = np.arange(128)[:, None]
    cc = np.arange(128)[None, :]
    tri = (cc >= kk).astype(bf)

    in_maps = []
    for core in range(8):
        b, g = core // 2, core % 2
        heads = range(8 * g, 8 * g + 8)
        rows = []
        for base in (0, C_, 2 * C_):  # q, k, v sections of w_attn
            for H in heads:
                rows.extend(range(base + 64 * H, base + 64 * H + 64))
        waT_l = np.ascontiguousarray(np.asarray(w_attn)[rows, :].T).astype(bf)
        dcols = [c for H in heads for c in range(64 * H, 64 * H + 64)]
        wpT_l = np.ascontiguousarray(np.asarray(w_proj)[:, dcols].T).astype(bf)
        xT_l = np.ascontiguousarray(np.asarray(x[b]).T).astype(bf)
        in_maps.append({"xT": xT_l, "waT": waT_l, "wpT": wpT_l, "tri": tri})

    global _LAST_IN_MAPS
    _LAST_IN_MAPS = in_maps
    res = bass_utils.run_bass_kernel_spmd(nc, in_maps, core_ids=list(range(8)))
    out = np.empty((B, T, C_), dtype=np.float32)
    for b in range(B):
        out[b] = (
            res.results[2 * b]["outT"].astype(np.float32)
            + res.results[2 * b + 1]["outT"].astype(np.float32)
        ).T
    return out


# revision 25
# speedup vs baseline: 1.5795x; 1.0729x over previous
"""Causal self-attention TRN2 kernel (8 NeuronCores, Megatron-style sharding).

Reference computation (fp32):
    qkv = x @ w_attn.T ; q,k,v split; per-head causal softmax(q k^T/sqrt(hs)) v
    out = y @ w_proj.T
Shapes: x [4, 2048, 1024], w_attn [3072, 1024], w_proj [1024, 1024], 16 heads.

Sharding: core = (b, g) with b = batch 0..3, g = head-group 0..1 (8 heads each).
Each core computes its batch's attention for its 8 heads plus the partial
output projection over its 512 local head-dims; host sums the two partials
per batch (Megatron row-parallel) and transposes back.

Device dataflow is fully transposed ([feature, token] layout) so the PE
contraction dim always sits on partitions with zero on-device transposes:
  qkT[d, t] = waT.T @ xT           (lhsT = waT block, rhs = xT)
  V[t, d]   = xT.T @ waT_v         (lhsT = xT block, rhs = wv)
  S.T[k, q] = KT.T @ QT            (lhsT = KT slice [hs=64, 128]; the two heads
                                    of a pair use PE row groups 0-1 / 2-3 and
                                    write the two halves of a [128,1024] PSUM)
  P = exp(S/8), computed only on causal columns (values are bounded, so no
      max-subtraction is needed); a [128,128] lower-tri mask fixes the
      diagonal block, zero-fill covers the below-diagonal prefix; softmax
      denominators come free from a ones column appended to V (the y.T
      matmul has M=65, row 64 = sum_k P)
  y.T[d, q] = V_aug.T @ P          (accumulated over k-tiles in PSUM)
  outT[e, q] = wpT.T @ yT          (partial over local d)
Matmuls run in bf16 (fp32 PSUM accumulation); softmax sums/normalization in
fp32.
"""

import math

import numpy as np

import concourse.bass as bass
import concourse.tile as tile
from concourse import bacc, mybir
from concourse import bass_utils

F32 = mybir.dt.float32
BF16 = mybir.dt.bfloat16
DT = BF16

C = 1024          # embed dim
NH_LOCAL = 8      # heads per core
HS = 64           # head size
DL = NH_LOCAL * HS  # local head-dim total (512)
NCT = C // 128    # c-tiles (contraction tiles) = 8


def build(T: int = 2048):
    """Build + compile the per-core program for sequence length T."""
    NQC = T // 512    # q-chunks
    NKT = T // 128    # k-tiles / t-tiles

    nc = bacc.Bacc(
        "TRN2", target_bir_lowering=False, debug=False, enable_asserts=False
    )

    xT = nc.dram_tensor("xT", [C, T], DT, kind="ExternalInput").ap()
    waT = nc.dram_tensor("waT", [C, 3 * DL], DT, kind="ExternalInput").ap()
    wpT = nc.dram_tensor("wpT", [DL, C], DT, kind="ExternalInput").ap()
    tri = nc.dram_tensor("tri", [128, 128], DT, kind="ExternalInput").ap()
    outT = nc.dram_tensor("outT", [C, T], F32, kind="ExternalOutput").ap()

    with tile.TileContext(nc) as tc:
        with (
            tc.tile_pool(name="const", bufs=1) as constp,
            tc.tile_pool(name="persist", bufs=1) as persist,
            tc.tile_pool(name="xtp", bufs=8) as xtp,
            tc.tile_pool(name="wblk", bufs=16) as wblkp,
            tc.tile_pool(name="wv", bufs=8) as wvp,
            tc.tile_pool(name="wpe", bufs=8) as wpep,
            tc.tile_pool(name="stage", bufs=4) as stagep,
            tc.tile_pool(name="epool", bufs=4) as epool,
            tc.tile_pool(name="misc", bufs=5) as miscp,
            tc.tile_pool(name="yup", bufs=18) as yup,
            tc.tile_pool(name="ps_small", bufs=2, space="PSUM") as ps_small,
            tc.tile_pool(name="ps_st", bufs=2, space="PSUM") as ps_st,
            tc.tile_pool(name="ps_yt", bufs=2, space="PSUM") as ps_yt,
        ):
            # ---- constants ----
            tri_t = constp.tile([128, 128], DT, tag="tri", name="tri_t")
            nc.sync.dma_start(tri_t[:], tri[:])

            # ---- persistent activations ----
            va_t = []  # V augmented with ones column: [128, 8*65]
            for tt in range(NKT):
                va = persist.tile(
                    [128, NH_LOCAL * (HS + 1)], DT, tag=f"va{tt}", name=f"va{tt}"
                )
                va_t.append(va)
            yt_t = []  # y.T per head-pair: [128, T]
            for p in range(4):
                yt = persist.tile([128, T], DT, tag=f"yt{p}", name=f"yt{p}")
                yt_t.append(yt)
            qk_t = []  # qkT resident: tiles 0-3 = QT pairs, 4-7 = KT pairs
            for dt in range(8):
                qk = persist.tile([128, T], DT, tag=f"qk{dt}", name=f"qk{dt}")
                qk_t.append(qk)

            # ================= phase A: projections =================
            xt_t = []
            for ci in range(NCT):
                xt = xtp.tile([128, T], DT, tag="xt", name=f"xt{ci}")
                nc.sync.dma_start(xt[:], xT[128 * ci : 128 * (ci + 1), :])
                xt_t.append(xt)

            # qkT = waT.T @ xT (resident in SBUF)
            for dt in range(8):
                wbs = []
                for ci in range(NCT):
                    wb = wblkp.tile([128, 128], DT, tag="wblk", name=f"wb{dt}_{ci}")
                    nc.sync.dma_start(
                        wb[:],
                        waT[128 * ci : 128 * (ci + 1), 128 * dt : 128 * (dt + 1)],
                    )
                    wbs.append(wb)
                for jq in range(T // 512):
                    ps = ps_small.tile([128, 512], F32, tag="psA", name="psA")
                    for ci in range(NCT):
                        nc.tensor.matmul(
                            ps[:],
                            wbs[ci][:],
                            xt_t[ci][:, 512 * jq : 512 * (jq + 1)],
                            start=(ci == 0),
                            stop=(ci == NCT - 1),
                        )
                    nc.vector.tensor_copy(
                        qk_t[dt][:, 512 * jq : 512 * (jq + 1)], ps[:]
                    )

            # V = xT.T @ wv  (+ ones column per head)
            wv_t = []
            for ci in range(NCT):
                wv = wvp.tile([128, DL], DT, tag="wv", name=f"wv{ci}")
                nc.sync.dma_start(wv[:], waT[128 * ci : 128 * (ci + 1), 2 * DL :])
                wv_t.append(wv)
            for tt in range(NKT):
                ps = ps_small.tile([128, 512], F32, tag="psA", name="psV")
                for ci in range(NCT):
                    nc.tensor.matmul(
                        ps[:],
                        xt_t[ci][:, 128 * tt : 128 * (tt + 1)],
                        wv_t[ci][:],
                        start=(ci == 0),
                        stop=(ci == NCT - 1),
                    )
                va = va_t[tt]
                va3 = va[:].rearrange("p (h d) -> p h d", d=HS + 1)
                ps3 = ps[:].rearrange("p (h d) -> p h d", d=HS)
                nc.vector.tensor_copy(va3[:, :, 0:HS], ps3[:])
                nc.vector.memset(va3[:, :, HS].bitcast(mybir.dt.uint16), 0x3F80)

            # ================= phase B: attention =================
            EXPF = mybir.ActivationFunctionType.Exp
            ISCALE = 1.0 / math.sqrt(HS)
            pending_norm = []
            for p in range(4):  # head pairs
                qt, kt = qk_t[p], qk_t[4 + p]
                # (j, h) softmax sums parked at 32-aligned partitions so one
                # [128,512] reciprocal covers 4 chunks at once
                NS = (2 * NQC + 3) // 4
                sums = [
                    miscp.tile([128, 512], F32, tag="sums", name=f"sums{s}")
                    for s in range(NS)
                ]
                rcs = [
                    miscp.tile([128, 512], F32, tag="rcs", name=f"rcs{s}")
                    for s in range(NS)
                ]
                for s in range(NS):
                    nc.vector.memset(sums[s][:], 1.0)
                yus = {}
                for j in range(NQC):
                    if j == 1 and pending_norm:
                        for fn in pending_norm:
                            fn()
                        pending_norm = []
                    qs = slice(512 * j, 512 * (j + 1))
                    ytps = [
                        ps_yt.tile([HS + 1, 512], F32, tag="ytp", name="ytp0"),
                        ps_yt.tile([HS + 1, 512], F32, tag="ytp", name="ytp1"),
                    ]
                    n_kt = 4 * j + 4
                    # Software pipeline: S.T/exp run two k-tiles ahead of the
                    # consuming y.T matmuls so the PE never waits on ACT.
                    LAG = 2
                    ets = {}
                    for i in range(n_kt + LAG):
                        if i < n_kt:
                            ks = slice(128 * i, 128 * (i + 1))
                            # both heads' S.T into one [128,1024] PSUM
                            st = ps_st.tile([128, 1024], F32, tag="stp", name="stp")
                            for h in range(2):
                                hp = slice(64 * h, 64 * (h + 1))
                                nc.tensor.matmul(
                                    st[:, 512 * h : 512 * (h + 1)],
                                    kt[hp, ks],
                                    qt[hp, qs],
                                    start=True,
                                    stop=True,
                                )
                            et = epool.tile([128, 1024], DT, tag="et", name="et")
                            o = 128 * (i - 4 * j)  # diag block offset, <0 if past
                            if o <= 0:  # fully causal k-tile
                                nc.scalar.activation(
                                    et[:], st[:], EXPF, scale=ISCALE
                                )
                            else:  # diagonal-crossing: skip invalid prefix
                                for h in range(2):
                                    c0 = 512 * h
                                    nc.vector.memset(
                                        et[:, c0 : c0 + o].bitcast(
                                            mybir.dt.uint16
                                        ),
                                        0,
                                    )
                                    nc.scalar.activation(
                                        et[:, c0 + o : c0 + 512],
                                        st[:, c0 + o : c0 + 512],
                                        EXPF,
                                        scale=ISCALE,
                                    )
                            if o >= 0:  # mask the diagonal 128-block per head
                                for h in range(2):
                                    db = slice(512 * h + o, 512 * h + o + 128)
                                    nc.vector.tensor_mul(
                                        et[:, db], et[:, db], tri_t[:]
                                    )
                            ets[i] = et
                        ic = i - LAG  # consume earlier k-tile
                        if ic >= 0:
                            et = ets.pop(ic)
                            for h in range(2):
                                hh = 2 * p + h
                                nc.tensor.matmul(
                                    ytps[h][:],
                                    va_t[ic][:, 65 * hh : 65 * hh + 65],
                                    et[:, 512 * h : 512 * (h + 1)],
                                    start=(ic == 0),
                                    stop=(ic == n_kt - 1),
                                )
                    for h in range(2):
                        # drain PSUM fast; park the sum row 32-aligned
                        yu = yup.tile([HS + 1, 512], F32, tag="yu", name="yu")
                        nc.vector.tensor_copy(yu[:], ytps[h][:])
                        yus[(j, h)] = yu
                        r = 2 * j + h
                        nc.vector.tensor_copy(
                            sums[r // 4][32 * (r % 4) : 32 * (r % 4) + 1, :],
                            yu[HS : HS + 1, :],
                        )
                # batched reciprocals + normalize, deferred one pair so
                # the DVE burst doesn't starve the next pair's mask ops
                def _normalize(p=p, sums=sums, rcs=rcs, yus=yus, NS=NS):
                    for s in range(NS):
                        nc.vector.reciprocal(rcs[s][:], sums[s][:])
                    for j in range(NQC):
                        qs = slice(512 * j, 512 * (j + 1))
                        for h in range(2):
                            r = 2 * j + h
                            # HW partition_broadcast reads partition 0 only:
                            # stage the reciprocal row there first
                            r0 = miscp.tile([1, 512], F32, tag="r0", name="r0")
                            nc.vector.tensor_copy(
                                r0[:],
                                rcs[r // 4][32 * (r % 4) : 32 * (r % 4) + 1, :],
                            )
                            rb = miscp.tile([64, 512], F32, tag="rb", name="rb")
                            nc.gpsimd.partition_broadcast(rb[:], r0[:])
                            nc.vector.tensor_mul(
                                yt_t[p][64 * h : 64 * (h + 1), qs],
                                yus.pop((j, h))[0:HS, :],
                                rb[:],
                            )

                pending_norm.append(_normalize)
            for fn in pending_norm:
                fn()
            pending_norm = []

            # ================= phase C: output projection =================
            for e in range(C // 128):
                wps = []
                for p in range(4):
                    wp = wpep.tile([128, 128], DT, tag="wpe", name=f"wpe{p}")
                    nc.sync.dma_start(
                        wp[:],
                        wpT[128 * p : 128 * (p + 1), 128 * e : 128 * (e + 1)],
                    )
                    wps.append(wp)
                for jq in range(T // 512):
                    ps = ps_small.tile([128, 512], F32, tag="psA", name="psC")
                    for p in range(4):
                        nc.tensor.matmul(
                            ps[:],
                            wps[p][:],
                            yt_t[p][:, 512 * jq : 512 * (jq + 1)],
                            start=(p == 0),
                            stop=(p == 3),
                        )
                    ot = stagep.tile([128, 512], F32, tag="stage", name="stC")
                    nc.scalar.copy(ot[:], ps[:])
                    nc.sync.dma_start(
                        outT[128 * e : 128 * (e + 1), 512 * jq : 512 * (jq + 1)],
                        ot[:],
                    )

    nc.compile()
    return nc


_CACHE: dict = {}
_LAST_IN_MAPS = None


def _get_nc(T: int):
    if T not in _CACHE:
        _CACHE[T] = build(T)
    return _CACHE[T]


def kernel(x: np.ndarray, w_attn: np.ndarray, w_proj: np.ndarray) -> np.ndarray:
    import ml_dtypes

    B, T, C_ = x.shape
    nc = _get_nc(T)
    bf = ml_dtypes.bfloat16
    kk = np.arange(128)[:, None]
    cc = np.arange(128)[None, :]
    tri = (cc >= kk).astype(bf)

    in_maps = []
    for core in range(8):
        b, g = core // 2, core % 2
        heads = range(8 * g, 8 * g + 8)
        rows = []
        for base in (0, C_, 2 * C_):  # q, k, v sections of w_attn
            for H in heads:
                rows.extend(range(base + 64 * H, base + 64 * H + 64))
        waT_l = np.ascontiguousarray(np.asarray(w_attn)[rows, :].T).astype(bf)
        dcols = [c for H in heads for c in range(64 * H, 64 * H + 64)]
        wpT_l = np.ascontiguousarray(np.asarray(w_proj)[:, dcols].T).astype(bf)
        xT_l = np.ascontiguousarray(np.asarray(x[b]).T).astype(bf)
        in_maps.append({"xT": xT_l, "waT": waT_l, "wpT": wpT_l, "tri": tri})

    global _LAST_IN_MAPS
    _LAST_IN_MAPS = in_maps
    res = bass_utils.run_bass_kernel_spmd(nc, in_maps, core_ids=list(range(8)))
    out = np.empty((B, T, C_), dtype=np.float32)
    for b in range(B):
        out[b] = (
            res.results[2 * b]["outT"].astype(np.float32)
            + res.results[2 * b + 1]["outT"].astype(np.float32)
        ).T
    return out


# revision 28
# speedup vs baseline: 1.6249x; 1.0288x over previous
"""Causal self-attention TRN2 kernel (8 NeuronCores, Megatron-style sharding).

Reference computation (fp32):
    qkv = x @ w_attn.T ; q,k,v split; per-head causal softmax(q k^T/sqrt(hs)) v
    out = y @ w_proj.T
Shapes: x [4, 2048, 1024], w_attn [3072, 1024], w_proj [1024, 1024], 16 heads.

Sharding: core = (b, g) with b = batch 0..3, g = head-group 0..1 (8 heads each).
Each core computes its batch's attention for its 8 heads plus the partial
output projection over its 512 local head-dims; host sums the two partials
per batch (Megatron row-parallel) and transposes back.

Device dataflow is fully transposed ([feature, token] layout) so the PE
contraction dim always sits on partitions with zero on-device transposes:
  qkT[d, t] = waT.T @ xT           (lhsT = waT block, rhs = xT)
  V[t, d]   = xT.T @ waT_v         (lhsT = xT block, rhs = wv)
  S.T[k, q] = KT.T @ QT            (lhsT = KT slice [hs=64, 128]; the two heads
                                    of a pair use PE row groups 0-1 / 2-3 and
                                    write the two halves of a [128,1024] PSUM)
  P = exp(S/8), computed only on causal columns (values are bounded, so no
      max-subtraction is needed); a [128,128] lower-tri mask fixes the
      diagonal block, zero-fill covers the below-diagonal prefix; softmax
      denominators come free from a ones column appended to V (the y.T
      matmul has M=65, row 64 = sum_k P)
  y.T[d, q] = V_aug.T @ P          (accumulated over k-tiles in PSUM)
  outT[e, q] = wpT.T @ yT          (partial over local d)
Matmuls run in bf16 (fp32 PSUM accumulation); softmax sums/normalization in
fp32.
"""

import math

import numpy as np

import concourse.bass as bass
import concourse.tile as tile
from concourse import bacc, mybir
from concourse import bass_utils

F32 = mybir.dt.float32
BF16 = mybir.dt.bfloat16
DT = BF16

C = 1024          # embed dim
NH_LOCAL = 8      # heads per core
HS = 64           # head size
DL = NH_LOCAL * HS  # local head-dim total (512)
NCT = C // 128    # c-tiles (contraction tiles) = 8


def build(T: int = 2048):
    """Build + compile the per-core program for sequence length T."""
    NQC = T // 512    # q-chunks
    NKT = T // 128    # k-tiles / t-tiles

    nc = bacc.Bacc(
        "TRN2", target_bir_lowering=False, debug=False, enable_asserts=False
    )

    xT = nc.dram_tensor("xT", [C, T], DT, kind="ExternalInput").ap()
    waT = nc.dram_tensor("waT", [C, 3 * DL], DT, kind="ExternalInput").ap()
    wpT = nc.dram_tensor("wpT", [DL, C], DT, kind="ExternalInput").ap()
    tri = nc.dram_tensor("tri", [128, 128], DT, kind="ExternalInput").ap()
    outT = nc.dram_tensor("outT", [C, T], F32, kind="ExternalOutput").ap()

    with tile.TileContext(nc) as tc:
        with (
            tc.tile_pool(name="const", bufs=1) as constp,
            tc.tile_pool(name="persist", bufs=1) as persist,
            tc.tile_pool(name="xtp", bufs=8) as xtp,
            tc.tile_pool(name="wblk", bufs=16) as wblkp,
            tc.tile_pool(name="wv", bufs=8) as wvp,
            tc.tile_pool(name="wpe", bufs=8) as wpep,
            tc.tile_pool(name="stage", bufs=4) as stagep,
            tc.tile_pool(name="epool", bufs=4) as epool,
            tc.tile_pool(name="misc", bufs=5) as miscp,
            tc.tile_pool(name="yup", bufs=18) as yup,
            tc.tile_pool(name="ps_small", bufs=2, space="PSUM") as ps_small,
            tc.tile_pool(name="ps_st", bufs=2, space="PSUM") as ps_st,
            tc.tile_pool(name="ps_yt", bufs=2, space="PSUM") as ps_yt,
        ):
            # ---- constants ----
            tri_t = constp.tile([128, 128], DT, tag="tri", name="tri_t")
            nc.sync.dma_start(tri_t[:], tri[:])

            # ---- persistent activations ----
            va_t = []  # V augmented with ones column: [128, 8*65]
            for tt in range(NKT):
                va = persist.tile(
                    [128, NH_LOCAL * (HS + 1)], DT, tag=f"va{tt}", name=f"va{tt}"
                )
                va_t.append(va)
            yt_t = []  # y.T per head-pair: [128, T]
            for p in range(4):
                yt = persist.tile([128, T], DT, tag=f"yt{p}", name=f"yt{p}")
                yt_t.append(yt)
            qk_t = []  # qkT resident: tiles 0-3 = QT pairs, 4-7 = KT pairs
            for dt in range(8):
                qk = persist.tile([128, T], DT, tag=f"qk{dt}", name=f"qk{dt}")
                qk_t.append(qk)

            # ================= phase A: projections =================
            xt_t = []
            for ci in range(NCT):
                xt = xtp.tile([128, T], DT, tag="xt", name=f"xt{ci}")
                nc.sync.dma_start(xt[:], xT[128 * ci : 128 * (ci + 1), :])
                xt_t.append(xt)

            wv_t = []
            for ci in range(NCT):
                wv = wvp.tile([128, DL], DT, tag="wv", name=f"wv{ci}")
                nc.sync.dma_start(wv[:], waT[128 * ci : 128 * (ci + 1), 2 * DL :])
                wv_t.append(wv)

            # qkT = waT.T @ xT (resident in SBUF)
            for dt in range(8):
                wbs = []
                for ci in range(NCT):
                    wb = wblkp.tile([128, 128], DT, tag="wblk", name=f"wb{dt}_{ci}")
                    nc.sync.dma_start(
                        wb[:],
                        waT[128 * ci : 128 * (ci + 1), 128 * dt : 128 * (dt + 1)],
                    )
                    wbs.append(wb)
                for jq in range(T // 512):
                    ps = ps_small.tile([128, 512], F32, tag="psA", name="psA")
                    for ci in range(NCT):
                        nc.tensor.matmul(
                            ps[:],
                            wbs[ci][:],
                            xt_t[ci][:, 512 * jq : 512 * (jq + 1)],
                            start=(ci == 0),
                            stop=(ci == NCT - 1),
                        )
                    nc.vector.tensor_copy(
                        qk_t[dt][:, 512 * jq : 512 * (jq + 1)], ps[:]
                    )

            # V = xT.T @ wv  (+ ones column per head)
            for tt in range(NKT):
                ps = ps_small.tile([128, 512], F32, tag="psA", name="psV")
                for ci in range(NCT):
                    nc.tensor.matmul(
                        ps[:],
                        xt_t[ci][:, 128 * tt : 128 * (tt + 1)],
                        wv_t[ci][:],
                        start=(ci == 0),
                        stop=(ci == NCT - 1),
                    )
                va = va_t[tt]
                va3 = va[:].rearrange("p (h d) -> p h d", d=HS + 1)
                ps3 = ps[:].rearrange("p (h d) -> p h d", d=HS)
                nc.vector.tensor_copy(va3[:, :, 0:HS], ps3[:])
                nc.vector.memset(va3[:, :, HS].bitcast(mybir.dt.uint16), 0x3F80)

            # ================= phase B: attention =================
            EXPF = mybir.ActivationFunctionType.Exp
            ISCALE = 1.0 / math.sqrt(HS)
            pending_norm = []
            for p in range(4):  # head pairs
                qt, kt = qk_t[p], qk_t[4 + p]
                # (j, h) softmax sums parked at 32-aligned partitions so one
                # [128,512] reciprocal covers 4 chunks at once
                NS = (2 * NQC + 3) // 4
                sums = [
                    miscp.tile([128, 512], F32, tag="sums", name=f"sums{s}")
                    for s in range(NS)
                ]
                rcs = [
                    miscp.tile([128, 512], F32, tag="rcs", name=f"rcs{s}")
                    for s in range(NS)
                ]
                for s in range(NS):
                    nc.vector.memset(sums[s][:], 1.0)
                yus = {}
                for j in range(NQC):
                    if j >= 1 and pending_norm:
                        take = 3 if j < NQC - 1 else len(pending_norm)
                        for fn in pending_norm[:take]:
                            fn()
                        pending_norm = pending_norm[take:]
                    qs = slice(512 * j, 512 * (j + 1))
                    ytps = [
                        ps_yt.tile([HS + 1, 512], F32, tag="ytp", name="ytp0"),
                        ps_yt.tile([HS + 1, 512], F32, tag="ytp", name="ytp1"),
                    ]
                    n_kt = 4 * j + 4
                    # Software pipeline: S.T/exp run two k-tiles ahead of the
                    # consuming y.T matmuls so the PE never waits on ACT.
                    LAG = 2
                    ets = {}
                    for i in range(n_kt + LAG):
                        if i < n_kt:
                            ks = slice(128 * i, 128 * (i + 1))
                            # both heads' S.T into one [128,1024] PSUM
                            st = ps_st.tile([128, 1024], F32, tag="stp", name="stp")
                            for h in range(2):
                                hp = slice(64 * h, 64 * (h + 1))
                                nc.tensor.matmul(
                                    st[:, 512 * h : 512 * (h + 1)],
                                    kt[hp, ks],
                                    qt[hp, qs],
                                    start=True,
                                    stop=True,
                                )
                            et = epool.tile([128, 1024], DT, tag="et", name="et")
                            o = 128 * (i - 4 * j)  # diag block offset, <0 if past
                            if o <= 0:  # fully causal k-tile
                                nc.scalar.activation(
                                    et[:], st[:], EXPF, scale=ISCALE
                                )
                            else:  # diagonal-crossing: skip invalid prefix
                                for h in range(2):
                                    c0 = 512 * h
                                    nc.scalar.activation(
                                        et[:, c0 + o : c0 + 512],
                                        st[:, c0 + o : c0 + 512],
                                        EXPF,
                                        scale=ISCALE,
                                    )
                            if o >= 0:  # mask the diagonal 128-block per head
                                for h in range(2):
                                    db = slice(512 * h + o, 512 * h + o + 128)
                                    nc.vector.tensor_mul(
                                        et[:, db], et[:, db], tri_t[:]
                                    )
                            ets[i] = (et, max(o, 0))
                        ic = i - LAG  # consume earlier k-tile
                        if ic >= 0:
                            et, o = ets.pop(ic)
                            for h in range(2):
                                hh = 2 * p + h
                                nc.tensor.matmul(
                                    ytps[h][:, o:512],
                                    va_t[ic][:, 65 * hh : 65 * hh + 65],
                                    et[:, 512 * h + o : 512 * (h + 1)],
                                    start=(ic == 0),
                                    stop=(ic == n_kt - 1),
                                )
                    for h in range(2):
                        # drain PSUM fast; park the sum row 32-aligned
                        yu = yup.tile([HS + 1, 512], F32, tag="yu", name="yu")
                        nc.vector.tensor_copy(yu[:], ytps[h][:])
                        yus[(j, h)] = yu
                        r = 2 * j + h
                        nc.vector.tensor_copy(
                            sums[r // 4][32 * (r % 4) : 32 * (r % 4) + 1, :],
                            yu[HS : HS + 1, :],
                        )
                # Normalization, deferred into the next pair's schedule so
                # the DVE burst doesn't starve the mask ops feeding the PE.
                # The last pair normalizes inline (phase C needs its yT).
                def _recip(p=p, sums=sums, rcs=rcs, NS=NS):
                    for s in range(NS):
                        nc.vector.reciprocal(rcs[s][:], sums[s][:])

                def _norm_one(j, h, p=p, rcs=rcs, yus=yus):
                    qs = slice(512 * j, 512 * (j + 1))
                    r = 2 * j + h
                    # HW partition_broadcast reads partition 0 only: stage
                    # the reciprocal row there first
                    r0 = miscp.tile([1, 512], F32, tag="r0", name="r0")
                    nc.vector.tensor_copy(
                        r0[:], rcs[r // 4][32 * (r % 4) : 32 * (r % 4) + 1, :]
                    )
                    rb = miscp.tile([64, 512], F32, tag="rb", name="rb")
                    nc.gpsimd.partition_broadcast(rb[:], r0[:])
                    nc.vector.tensor_mul(
                        yt_t[p][64 * h : 64 * (h + 1), qs],
                        yus.pop((j, h))[0:HS, :],
                        rb[:],
                    )

                units = [_recip] + [
                    (lambda j=j, h=h, f=_norm_one: f(j, h))
                    for j in range(NQC)
                    for h in range(2)
                ]
                if p == 3:
                    for fn in units:
                        fn()
                else:
                    pending_norm.extend(units)
            for fn in pending_norm:
                fn()
            pending_norm = []

            # ================= phase C: output projection =================
            for e in range(C // 128):
                wps = []
                for p in range(4):
                    wp = wpep.tile([128, 128], DT, tag="wpe", name=f"wpe{p}")
                    nc.sync.dma_start(
                        wp[:],
                        wpT[128 * p : 128 * (p + 1), 128 * e : 128 * (e + 1)],
                    )
                    wps.append(wp)
                for jq in range(T // 512):
                    ps = ps_small.tile([128, 512], F32, tag="psA", name="psC")
                    for p in range(4):
                        nc.tensor.matmul(
                            ps[:],
                            wps[p][:],
                            yt_t[p][:, 512 * jq : 512 * (jq + 1)],
                            start=(p == 0),
                            stop=(p == 3),
                        )
                    ot = stagep.tile([128, 512], F32, tag="stage", name="stC")
                    nc.scalar.copy(ot[:], ps[:])
                    nc.sync.dma_start(
                        outT[128 * e : 128 * (e + 1), 512 * jq : 512 * (jq + 1)],
                        ot[:],
                    )

    nc.compile()
    return nc


_CACHE: dict = {}
_LAST_IN_MAPS = None


def _get_nc(T: int):
    if T not in _CACHE:
        _CACHE[T] = build(T)
    return _CACHE[T]


def kernel(x: np.ndarray, w_attn: np.ndarray, w_proj: np.ndarray) -> np.ndarray:
    import ml_dtypes

    B, T, C_ = x.shape
    nc = _get_nc(T)
    bf = ml_dtypes.bfloat16
    kk = np.arange(128)[:, None]
    cc = np.arange(128)[None, :]
    tri = (cc >= kk).astype(bf)

    in_maps = []
    for core in range(8):
        b, g = core // 2, core % 2
        heads = range(8 * g, 8 * g + 8)
        rows = []
        for base in (0, C_, 2 * C_):  # q, k, v sections of w_attn
            for H in heads:
                rows.extend(range(base + 64 * H, base + 64 * H + 64))
        waT_l = np.ascontiguousarray(np.asarray(w_attn)[rows, :].T).astype(bf)
        dcols = [c for H in heads for c in range(64 * H, 64 * H + 64)]
        wpT_l = np.ascontiguousarray(np.asarray(w_proj)[:, dcols].T).astype(bf)
        xT_l = np.ascontiguousarray(np.asarray(x[b]).T).astype(bf)
        in_maps.append({"xT": xT_l, "waT": waT_l, "wpT": wpT_l, "tri": tri})

    global _LAST_IN_MAPS
    _LAST_IN_MAPS = in_maps
    res = bass_utils.run_bass_kernel_spmd(nc, in_maps, core_ids=list(range(8)))
    out = np.empty((B, T, C_), dtype=np.float32)
    for b in range(B):
        out[b] = (
            res.results[2 * b]["outT"].astype(np.float32)
            + res.results[2 * b + 1]["outT"].astype(np.float32)
        ).T
    return out


# revision 29
# speedup vs baseline: 1.6494x; 1.0151x over previous
"""Causal self-attention TRN2 kernel (8 NeuronCores, Megatron-style sharding).

Reference computation (fp32):
    qkv = x @ w_attn.T ; q,k,v split; per-head causal softmax(q k^T/sqrt(hs)) v
    out = y @ w_proj.T
Shapes: x [4, 2048, 1024], w_attn [3072, 1024], w_proj [1024, 1024], 16 heads.

Sharding: core = (b, g) with b = batch 0..3, g = head-group 0..1 (8 heads each).
Each core computes its batch's attention for its 8 heads plus the partial
output projection over its 512 local head-dims; host sums the two partials
per batch (Megatron row-parallel) and transposes back.

Device dataflow is fully transposed ([feature, token] layout) so the PE
contraction dim always sits on partitions with zero on-device transposes:
  qkT[d, t] = waT.T @ xT           (lhsT = waT block, rhs = xT)
  V[t, d]   = xT.T @ waT_v         (lhsT = xT block, rhs = wv)
  S.T[k, q] = KT.T @ QT            (lhsT = KT slice [hs=64, 128]; the two heads
                                    of a pair use PE row groups 0-1 / 2-3 and
                                    write the two halves of a [128,1024] PSUM)
  P = exp(S/8), computed only on causal columns (values are bounded, so no
      max-subtraction is needed); a [128,128] lower-tri mask fixes the
      diagonal block, zero-fill covers the below-diagonal prefix; softmax
      denominators come free from a ones column appended to V (the y.T
      matmul has M=65, row 64 = sum_k P)
  y.T[d, q] = V_aug.T @ P          (accumulated over k-tiles in PSUM)
  outT[e, q] = wpT.T @ yT          (partial over local d)
Matmuls run in bf16 (fp32 PSUM accumulation); softmax sums/normalization in
fp32.
"""

import math

import numpy as np

import concourse.bass as bass
import concourse.tile as tile
from concourse import bacc, mybir
from concourse import bass_utils

F32 = mybir.dt.float32
BF16 = mybir.dt.bfloat16
DT = BF16

C = 1024          # embed dim
NH_LOCAL = 8      # heads per core
HS = 64           # head size
DL = NH_LOCAL * HS  # local head-dim total (512)
NCT = C // 128    # c-tiles (contraction tiles) = 8


def build(T: int = 2048):
    """Build + compile the per-core program for sequence length T."""
    NQC = T // 512    # q-chunks
    NKT = T // 128    # k-tiles / t-tiles

    nc = bacc.Bacc(
        "TRN2", target_bir_lowering=False, debug=False, enable_asserts=False
    )

    xT = nc.dram_tensor("xT", [C, T], DT, kind="ExternalInput").ap()
    waT = nc.dram_tensor("waT", [C, 3 * DL], DT, kind="ExternalInput").ap()
    wpT = nc.dram_tensor("wpT", [DL, C], DT, kind="ExternalInput").ap()
    tri = nc.dram_tensor("tri", [128, 128], DT, kind="ExternalInput").ap()
    outT = nc.dram_tensor("outT", [C, T], F32, kind="ExternalOutput").ap()

    with tile.TileContext(nc) as tc:
        with (
            tc.tile_pool(name="const", bufs=1) as constp,
            tc.tile_pool(name="persist", bufs=1) as persist,
            tc.tile_pool(name="xtp", bufs=8) as xtp,
            tc.tile_pool(name="wblk", bufs=16) as wblkp,
            tc.tile_pool(name="wv", bufs=8) as wvp,
            tc.tile_pool(name="wpe", bufs=8) as wpep,
            tc.tile_pool(name="stage", bufs=4) as stagep,
            tc.tile_pool(name="epool", bufs=4) as epool,
            tc.tile_pool(name="misc", bufs=5) as miscp,
            tc.tile_pool(name="yup", bufs=18) as yup,
            tc.tile_pool(name="ps_small", bufs=2, space="PSUM") as ps_small,
            tc.tile_pool(name="ps_st", bufs=2, space="PSUM") as ps_st,
            tc.tile_pool(name="ps_yt", bufs=2, space="PSUM") as ps_yt,
        ):
            # ---- constants ----
            tri_t = constp.tile([128, 128], DT, tag="tri", name="tri_t")
            nc.sync.dma_start(tri_t[:], tri[:])

            # ---- persistent activations ----
            va_t = []  # V augmented with ones column: [128, 8*65]
            for tt in range(NKT):
                va = persist.tile(
                    [128, NH_LOCAL * (HS + 1)], DT, tag=f"va{tt}", name=f"va{tt}"
                )
                va_t.append(va)
            yt_t = []  # y.T per head-pair: [128, T]
            for p in range(4):
                yt = persist.tile([128, T], DT, tag=f"yt{p}", name=f"yt{p}")
                yt_t.append(yt)
            qk_t = []  # qkT resident: tiles 0-3 = QT pairs, 4-7 = KT pairs
            for dt in range(8):
                qk = persist.tile([128, T], DT, tag=f"qk{dt}", name=f"qk{dt}")
                qk_t.append(qk)

            # ================= phase A: projections =================
            xt_t = []
            for ci in range(NCT):
                xt = xtp.tile([128, T], DT, tag="xt", name=f"xt{ci}")
                nc.sync.dma_start(xt[:], xT[128 * ci : 128 * (ci + 1), :])
                xt_t.append(xt)

            wv_t = []
            for ci in range(NCT):
                wv = wvp.tile([128, DL], DT, tag="wv", name=f"wv{ci}")
                nc.sync.dma_start(wv[:], waT[128 * ci : 128 * (ci + 1), 2 * DL :])
                wv_t.append(wv)

            # qkT = waT.T @ xT (resident in SBUF)
            for dt in range(8):
                wbs = []
                for ci in range(NCT):
                    wb = wblkp.tile([128, 128], DT, tag="wblk", name=f"wb{dt}_{ci}")
                    nc.sync.dma_start(
                        wb[:],
                        waT[128 * ci : 128 * (ci + 1), 128 * dt : 128 * (dt + 1)],
                    )
                    wbs.append(wb)
                for jq in range(T // 512):
                    ps = ps_small.tile([128, 512], F32, tag="psA", name="psA")
                    for ci in range(NCT):
                        nc.tensor.matmul(
                            ps[:],
                            wbs[ci][:],
                            xt_t[ci][:, 512 * jq : 512 * (jq + 1)],
                            start=(ci == 0),
                            stop=(ci == NCT - 1),
                        )
                    nc.vector.tensor_copy(
                        qk_t[dt][:, 512 * jq : 512 * (jq + 1)], ps[:]
                    )

            # V = xT.T @ wv  (+ ones column per head)
            for tt in range(NKT):
                ps = ps_small.tile([128, 512], F32, tag="psA", name="psV")
                for ci in range(NCT):
                    nc.tensor.matmul(
                        ps[:],
                        xt_t[ci][:, 128 * tt : 128 * (tt + 1)],
                        wv_t[ci][:],
                        start=(ci == 0),
                        stop=(ci == NCT - 1),
                    )
                va = va_t[tt]
                va3 = va[:].rearrange("p (h d) -> p h d", d=HS + 1)
                ps3 = ps[:].rearrange("p (h d) -> p h d", d=HS)
                nc.vector.tensor_copy(va3[:, :, 0:HS], ps3[:])
                nc.vector.memset(va3[:, :, HS].bitcast(mybir.dt.uint16), 0x3F80)

            # ================= phase B: attention =================
            EXPF = mybir.ActivationFunctionType.Exp
            ISCALE = 1.0 / math.sqrt(HS)
            pending_norm = []
            for p in range(4):  # head pairs
                qt, kt = qk_t[p], qk_t[4 + p]
                # (j, h) softmax sums parked at 32-aligned partitions so one
                # [128,512] reciprocal covers 4 chunks at once
                NS = (2 * NQC + 3) // 4
                sums = [
                    miscp.tile([128, 512], F32, tag="sums", name=f"sums{s}")
                    for s in range(NS)
                ]
                rcs = [
                    miscp.tile([128, 512], F32, tag="rcs", name=f"rcs{s}")
                    for s in range(NS)
                ]
                for s in range(NS):
                    nc.vector.memset(sums[s][:], 1.0)
                yus = {}
                for j in range(NQC):
                    if j >= 2 and pending_norm:
                        take = 4 if j < NQC - 1 else len(pending_norm)
                        for fn in pending_norm[:take]:
                            fn()
                        pending_norm = pending_norm[take:]
                    qs = slice(512 * j, 512 * (j + 1))
                    ytps = [
                        ps_yt.tile([HS + 1, 512], F32, tag="ytp", name="ytp0"),
                        ps_yt.tile([HS + 1, 512], F32, tag="ytp", name="ytp1"),
                    ]
                    n_kt = 4 * j + 4
                    # Software pipeline: S.T/exp run two k-tiles ahead of the
                    # consuming y.T matmuls so the PE never waits on ACT.
                    LAG = 2
                    ets = {}
                    for i in range(n_kt + LAG):
                        if i < n_kt:
                            ks = slice(128 * i, 128 * (i + 1))
                            # both heads' S.T into one [128,1024] PSUM
                            st = ps_st.tile([128, 1024], F32, tag="stp", name="stp")
                            for h in range(2):
                                hp = slice(64 * h, 64 * (h + 1))
                                nc.tensor.matmul(
                                    st[:, 512 * h : 512 * (h + 1)],
                                    kt[hp, ks],
                                    qt[hp, qs],
                                    start=True,
                                    stop=True,
                                )
                            et = epool.tile([128, 1024], DT, tag="et", name="et")
                            o = 128 * (i - 4 * j)  # diag block offset, <0 if past
                            if o <= 0:  # fully causal k-tile
                                nc.scalar.activation(
                                    et[:], st[:], EXPF, scale=ISCALE
                                )
                            else:  # diagonal-crossing: skip invalid prefix
                                for h in range(2):
                                    c0 = 512 * h
                                    nc.scalar.activation(
                                        et[:, c0 + o : c0 + 512],
                                        st[:, c0 + o : c0 + 512],
                                        EXPF,
                                        scale=ISCALE,
                                    )
                            if o >= 0:  # mask the diagonal 128-block per head
                                for h in range(2):
                                    db = slice(512 * h + o, 512 * h + o + 128)
                                    nc.vector.tensor_mul(
                                        et[:, db], et[:, db], tri_t[:]
                                    )
                            ets[i] = (et, max(o, 0))
                        ic = i - LAG  # consume earlier k-tile
                        if ic >= 0:
                            et, o = ets.pop(ic)
                            for h in range(2):
                                hh = 2 * p + h
                                nc.tensor.matmul(
                                    ytps[h][:, o:512],
                                    va_t[ic][:, 65 * hh : 65 * hh + 65],
                                    et[:, 512 * h + o : 512 * (h + 1)],
                                    start=(ic == 0),
                                    stop=(ic == n_kt - 1),
                                )
                    for h in range(2):
                        # drain PSUM fast; park the sum row 32-aligned
                        yu = yup.tile([HS + 1, 512], F32, tag="yu", name="yu")
                        nc.vector.tensor_copy(yu[:], ytps[h][:])
                        yus[(j, h)] = yu
                        r = 2 * j + h
                        nc.vector.tensor_copy(
                            sums[r // 4][32 * (r % 4) : 32 * (r % 4) + 1, :],
                            yu[HS : HS + 1, :],
                        )
                # Normalization, deferred into the next pair's schedule so
                # the DVE burst doesn't starve the mask ops feeding the PE.
                # The last pair normalizes inline (phase C needs its yT).
                def _recip(p=p, sums=sums, rcs=rcs, NS=NS):
                    for s in range(NS):
                        nc.vector.reciprocal(rcs[s][:], sums[s][:])

                def _norm_one(j, h, p=p, rcs=rcs, yus=yus):
                    qs = slice(512 * j, 512 * (j + 1))
                    r = 2 * j + h
                    # HW partition_broadcast reads partition 0 only: stage
                    # the reciprocal row there first
                    r0 = miscp.tile([1, 512], F32, tag="r0", name="r0")
                    nc.vector.tensor_copy(
                        r0[:], rcs[r // 4][32 * (r % 4) : 32 * (r % 4) + 1, :]
                    )
                    rb = miscp.tile([64, 512], F32, tag="rb", name="rb")
                    nc.gpsimd.partition_broadcast(rb[:], r0[:])
                    nc.vector.tensor_mul(
                        yt_t[p][64 * h : 64 * (h + 1), qs],
                        yus.pop((j, h))[0:HS, :],
                        rb[:],
                    )

                units = [_recip] + [
                    (lambda j=j, h=h, f=_norm_one: f(j, h))
                    for j in range(NQC)
                    for h in range(2)
                ]
                if p == 3:
                    for fn in units:
                        fn()
                else:
                    pending_norm.extend(units)
            for fn in pending_norm:
                fn()
            pending_norm = []

            # ================= phase C: output projection =================
            for e in range(C // 128):
                wps = []
                for p in range(4):
                    wp = wpep.tile([128, 128], DT, tag="wpe", name=f"wpe{p}")
                    nc.sync.dma_start(
                        wp[:],
                        wpT[128 * p : 128 * (p + 1), 128 * e : 128 * (e + 1)],
                    )
                    wps.append(wp)
                for jq in range(T // 512):
                    ps = ps_small.tile([128, 512], F32, tag="psA", name="psC")
                    for p in range(4):
                        nc.tensor.matmul(
                            ps[:],
                            wps[p][:],
                            yt_t[p][:, 512 * jq : 512 * (jq + 1)],
                            start=(p == 0),
                            stop=(p == 3),
                        )
                    ot = stagep.tile([128, 512], F32, tag="stage", name="stC")
                    nc.vector.tensor_copy(ot[:], ps[:])
                    nc.sync.dma_start(
                        outT[128 * e : 128 * (e + 1), 512 * jq : 512 * (jq + 1)],
                        ot[:],
                    )

    nc.compile()
    return nc


_CACHE: dict = {}
_LAST_IN_MAPS = None


def _get_nc(T: int):
    if T not in _CACHE:
        _CACHE[T] = build(T)
    return _CACHE[T]


def kernel(x: np.ndarray, w_attn: np.ndarray, w_proj: np.ndarray) -> np.ndarray:
    import ml_dtypes

    B, T, C_ = x.shape
    nc = _get_nc(T)
    bf = ml_dtypes.bfloat16
    kk = np.arange(128)[:, None]
    cc = np.arange(128)[None, :]
    tri = (cc >= kk).astype(bf)

    in_maps = []
    for core in range(8):
        b, g = core // 2, core % 2
        heads = range(8 * g, 8 * g + 8)
        rows = []
        for base in (0, C_, 2 * C_):  # q, k, v sections of w_attn
            for H in heads:
                rows.extend(range(base + 64 * H, base + 64 * H + 64))
        waT_l = np.ascontiguousarray(np.asarray(w_attn)[rows, :].T).astype(bf)
        dcols = [c for H in heads for c in range(64 * H, 64 * H + 64)]
        wpT_l = np.ascontiguousarray(np.asarray(w_proj)[:, dcols].T).astype(bf)
        xT_l = np.ascontiguousarray(np.asarray(x[b]).T).astype(bf)
        in_maps.append({"xT": xT_l, "waT": waT_l, "wpT": wpT_l, "tri": tri})

    global _LAST_IN_MAPS
    _LAST_IN_MAPS = in_maps
    res = bass_utils.run_bass_kernel_spmd(nc, in_maps, core_ids=list(range(8)))
    out = np.empty((B, T, C_), dtype=np.float32)
    for b in range(B):
        out[b] = (
            res.results[2 * b]["outT"].astype(np.float32)
            + res.results[2 * b + 1]["outT"].astype(np.float32)
        ).T
    return out


# revision 30
# speedup vs baseline: 1.6515x; 1.0013x over previous
"""Causal self-attention TRN2 kernel (8 NeuronCores, Megatron-style sharding).

Reference computation (fp32):
    qkv = x @ w_attn.T ; q,k,v split; per-head causal softmax(q k^T/sqrt(hs)) v
    out = y @ w_proj.T
Shapes: x [4, 2048, 1024], w_attn [3072, 1024], w_proj [1024, 1024], 16 heads.

Sharding: core = (b, g) with b = batch 0..3, g = head-group 0..1 (8 heads each).
Each core computes its batch's attention for its 8 heads plus the partial
output projection over its 512 local head-dims; host sums the two partials
per batch (Megatron row-parallel) and transposes back.

Device dataflow is fully transposed ([feature, token] layout) so the PE
contraction dim always sits on partitions with zero on-device transposes:
  qkT[d, t] = waT.T @ xT           (lhsT = waT block, rhs = xT)
  V[t, d]   = xT.T @ waT_v         (lhsT = xT block, rhs = wv)
  S.T[k, q] = KT.T @ QT            (lhsT = KT slice [hs=64, 128]; the two heads
                                    of a pair use PE row groups 0-1 / 2-3 and
                                    write the two halves of a [128,1024] PSUM)
  P = exp(S/8), computed only on causal columns (values are bounded, so no
      max-subtraction is needed); a [128,128] lower-tri mask fixes the
      diagonal block, zero-fill covers the below-diagonal prefix; softmax
      denominators come free from a ones column appended to V (the y.T
      matmul has M=65, row 64 = sum_k P)
  y.T[d, q] = V_aug.T @ P          (accumulated over k-tiles in PSUM)
  outT[e, q] = wpT.T @ yT          (partial over local d)
Matmuls run in bf16 (fp32 PSUM accumulation); softmax sums/normalization in
fp32.
"""

import math

import numpy as np

import concourse.bass as bass
import concourse.tile as tile
from concourse import bacc, mybir
from concourse import bass_utils

F32 = mybir.dt.float32
BF16 = mybir.dt.bfloat16
DT = BF16

C = 1024          # embed dim
NH_LOCAL = 8      # heads per core
HS = 64           # head size
DL = NH_LOCAL * HS  # local head-dim total (512)
NCT = C // 128    # c-tiles (contraction tiles) = 8


def build(T: int = 2048):
    """Build + compile the per-core program for sequence length T."""
    NQC = T // 512    # q-chunks
    NKT = T // 128    # k-tiles / t-tiles

    nc = bacc.Bacc(
        "TRN2", target_bir_lowering=False, debug=False, enable_asserts=False
    )

    xT = nc.dram_tensor("xT", [C, T], DT, kind="ExternalInput").ap()
    waT = nc.dram_tensor("waT", [C, 3 * DL], DT, kind="ExternalInput").ap()
    wpT = nc.dram_tensor("wpT", [DL, C], DT, kind="ExternalInput").ap()
    tri = nc.dram_tensor("tri", [128, 128], DT, kind="ExternalInput").ap()
    outT = nc.dram_tensor("outT", [C, T], F32, kind="ExternalOutput").ap()

    with tile.TileContext(nc) as tc:
        with (
            tc.tile_pool(name="const", bufs=1) as constp,
            tc.tile_pool(name="persist", bufs=1) as persist,
            tc.tile_pool(name="xtp", bufs=8) as xtp,
            tc.tile_pool(name="wblk", bufs=16) as wblkp,
            tc.tile_pool(name="wv", bufs=8) as wvp,
            tc.tile_pool(name="wpe", bufs=8) as wpep,
            tc.tile_pool(name="stage", bufs=4) as stagep,
            tc.tile_pool(name="epool", bufs=6) as epool,
            tc.tile_pool(name="misc", bufs=5) as miscp,
            tc.tile_pool(name="yup", bufs=18) as yup,
            tc.tile_pool(name="ps_small", bufs=2, space="PSUM") as ps_small,
            tc.tile_pool(name="ps_st", bufs=2, space="PSUM") as ps_st,
            tc.tile_pool(name="ps_yt", bufs=2, space="PSUM") as ps_yt,
        ):
            # ---- constants ----
            tri_t = constp.tile([128, 128], DT, tag="tri", name="tri_t")
            nc.sync.dma_start(tri_t[:], tri[:])

            # ---- persistent activations ----
            va_t = []  # V augmented with ones column: [128, 8*65]
            for tt in range(NKT):
                va = persist.tile(
                    [128, NH_LOCAL * (HS + 1)], DT, tag=f"va{tt}", name=f"va{tt}"
                )
                va_t.append(va)
            yt_t = []  # y.T per head-pair: [128, T]
            for p in range(4):
                yt = persist.tile([128, T], DT, tag=f"yt{p}", name=f"yt{p}")
                yt_t.append(yt)
            qk_t = []  # qkT resident: tiles 0-3 = QT pairs, 4-7 = KT pairs
            for dt in range(8):
                qk = persist.tile([128, T], DT, tag=f"qk{dt}", name=f"qk{dt}")
                qk_t.append(qk)

            # ================= phase A: projections =================
            xt_t = []

            # qkT = waT.T @ xT (resident in SBUF). First d-tile's weight
            # blocks are loaded before everything else so the PE can start
            # as soon as the xT stream lands.
            all_wbs = {}
            for dt in range(8):
                wbs = []
                for ci in range(NCT):
                    wb = wblkp.tile([128, 128], DT, tag="wblk", name=f"wb{dt}_{ci}")
                    nc.sync.dma_start(
                        wb[:],
                        waT[128 * ci : 128 * (ci + 1), 128 * dt : 128 * (dt + 1)],
                    )
                    wbs.append(wb)
                all_wbs[dt] = wbs
                if dt == 0:
                    for ci in range(NCT):
                        xt = xtp.tile([128, T], DT, tag="xt", name=f"xt{ci}")
                        nc.sync.dma_start(
                            xt[:], xT[128 * ci : 128 * (ci + 1), :]
                        )
                        xt_t.append(xt)
            wv_t = []
            for ci in range(NCT):
                wv = wvp.tile([128, DL], DT, tag="wv", name=f"wv{ci}")
                nc.sync.dma_start(wv[:], waT[128 * ci : 128 * (ci + 1), 2 * DL :])
                wv_t.append(wv)
            for dt in range(8):
                wbs = all_wbs[dt]
                for jq in range(T // 512):
                    ps = ps_small.tile([128, 512], F32, tag="psA", name="psA")
                    for ci in range(NCT):
                        nc.tensor.matmul(
                            ps[:],
                            wbs[ci][:],
                            xt_t[ci][:, 512 * jq : 512 * (jq + 1)],
                            start=(ci == 0),
                            stop=(ci == NCT - 1),
                        )
                    nc.vector.tensor_copy(
                        qk_t[dt][:, 512 * jq : 512 * (jq + 1)], ps[:]
                    )

            # V = xT.T @ wv  (+ ones column per head)
            for tt in range(NKT):
                ps = ps_small.tile([128, 512], F32, tag="psA", name="psV")
                for ci in range(NCT):
                    nc.tensor.matmul(
                        ps[:],
                        xt_t[ci][:, 128 * tt : 128 * (tt + 1)],
                        wv_t[ci][:],
                        start=(ci == 0),
                        stop=(ci == NCT - 1),
                    )
                va = va_t[tt]
                va3 = va[:].rearrange("p (h d) -> p h d", d=HS + 1)
                ps3 = ps[:].rearrange("p (h d) -> p h d", d=HS)
                nc.vector.tensor_copy(va3[:, :, 0:HS], ps3[:])
                nc.vector.memset(va3[:, :, HS].bitcast(mybir.dt.uint16), 0x3F80)

            # ================= phase B: attention =================
            EXPF = mybir.ActivationFunctionType.Exp
            ISCALE = 1.0 / math.sqrt(HS)
            pending_norm = []
            for p in range(4):  # head pairs
                qt, kt = qk_t[p], qk_t[4 + p]
                # (j, h) softmax sums parked at 32-aligned partitions so one
                # [128,512] reciprocal covers 4 chunks at once
                NS = (2 * NQC + 3) // 4
                sums = [
                    miscp.tile([128, 512], F32, tag="sums", name=f"sums{s}")
                    for s in range(NS)
                ]
                rcs = [
                    miscp.tile([128, 512], F32, tag="rcs", name=f"rcs{s}")
                    for s in range(NS)
                ]
                for s in range(NS):
                    nc.vector.memset(sums[s][:], 1.0)
                yus = {}
                for j in range(NQC):
                    if j >= 2 and pending_norm:
                        take = 4 if j < NQC - 1 else len(pending_norm)
                        for fn in pending_norm[:take]:
                            fn()
                        pending_norm = pending_norm[take:]
                    qs = slice(512 * j, 512 * (j + 1))
                    ytps = [
                        ps_yt.tile([HS + 1, 512], F32, tag="ytp", name="ytp0"),
                        ps_yt.tile([HS + 1, 512], F32, tag="ytp", name="ytp1"),
                    ]
                    n_kt = 4 * j + 4
                    # Software pipeline: S.T/exp run two k-tiles ahead of the
                    # consuming y.T matmuls so the PE never waits on ACT.
                    LAG = 3
                    ets = {}
                    for i in range(n_kt + LAG):
                        if i < n_kt:
                            ks = slice(128 * i, 128 * (i + 1))
                            # both heads' S.T into one [128,1024] PSUM
                            st = ps_st.tile([128, 1024], F32, tag="stp", name="stp")
                            for h in range(2):
                                hp = slice(64 * h, 64 * (h + 1))
                                nc.tensor.matmul(
                                    st[:, 512 * h : 512 * (h + 1)],
                                    kt[hp, ks],
                                    qt[hp, qs],
                                    start=True,
                                    stop=True,
                                )
                            et = epool.tile([128, 1024], DT, tag="et", name="et")
                            o = 128 * (i - 4 * j)  # diag block offset, <0 if past
                            if o <= 0:  # fully causal k-tile
                                nc.scalar.activation(
                                    et[:], st[:], EXPF, scale=ISCALE
                                )
                            else:  # diagonal-crossing: skip invalid prefix
                                for h in range(2):
                                    c0 = 512 * h
                                    nc.scalar.activation(
                                        et[:, c0 + o : c0 + 512],
                                        st[:, c0 + o : c0 + 512],
                                        EXPF,
                                        scale=ISCALE,
                                    )
                            if o >= 0:  # mask the diagonal 128-block per head
                                for h in range(2):
                                    db = slice(512 * h + o, 512 * h + o + 128)
                                    nc.vector.tensor_mul(
                                        et[:, db], et[:, db], tri_t[:]
                                    )
                            ets[i] = (et, max(o, 0))
                        ic = i - LAG  # consume earlier k-tile
                        if ic >= 0:
                            et, o = ets.pop(ic)
                            for h in range(2):
                                hh = 2 * p + h
                                nc.tensor.matmul(
                                    ytps[h][:, o:512],
                                    va_t[ic][:, 65 * hh : 65 * hh + 65],
                                    et[:, 512 * h + o : 512 * (h + 1)],
                                    start=(ic == 0),
                                    stop=(ic == n_kt - 1),
                                )
                    for h in range(2):
                        # drain PSUM fast; park the sum row 32-aligned
                        yu = yup.tile([HS + 1, 512], F32, tag="yu", name="yu")
                        nc.vector.tensor_copy(yu[:], ytps[h][:])
                        yus[(j, h)] = yu
                        r = 2 * j + h
                        nc.vector.tensor_copy(
                            sums[r // 4][32 * (r % 4) : 32 * (r % 4) + 1, :],
                            yu[HS : HS + 1, :],
                        )
                # Normalization, deferred into the next pair's schedule so
                # the DVE burst doesn't starve the mask ops feeding the PE.
                # The last pair normalizes inline (phase C needs its yT).
                def _recip(p=p, sums=sums, rcs=rcs, NS=NS):
                    for s in range(NS):
                        nc.vector.reciprocal(rcs[s][:], sums[s][:])

                def _norm_one(j, h, p=p, rcs=rcs, yus=yus):
                    qs = slice(512 * j, 512 * (j + 1))
                    r = 2 * j + h
                    # HW partition_broadcast reads partition 0 only: stage
                    # the reciprocal row there first
                    r0 = miscp.tile([1, 512], F32, tag="r0", name="r0")
                    nc.vector.tensor_copy(
                        r0[:], rcs[r // 4][32 * (r % 4) : 32 * (r % 4) + 1, :]
                    )
                    rb = miscp.tile([64, 512], F32, tag="rb", name="rb")
                    nc.gpsimd.partition_broadcast(rb[:], r0[:])
                    nc.vector.tensor_mul(
                        yt_t[p][64 * h : 64 * (h + 1), qs],
                        yus.pop((j, h))[0:HS, :],
                        rb[:],
                    )

                units = [_recip] + [
                    (lambda j=j, h=h, f=_norm_one: f(j, h))
                    for j in range(NQC)
                    for h in range(2)
                ]
                if p == 3:
                    for fn in units:
                        fn()
                else:
                    pending_norm.extend(units)
            for fn in pending_norm:
                fn()
            pending_norm = []

            # ================= phase C: output projection =================
            for e in range(C // 128):
                wps = []
                for p in range(4):
                    wp = wpep.tile([128, 128], DT, tag="wpe", name=f"wpe{p}")
                    nc.sync.dma_start(
                        wp[:],
                        wpT[128 * p : 128 * (p + 1), 128 * e : 128 * (e + 1)],
                    )
                    wps.append(wp)
                for jq in range(T // 512):
                    ps = ps_small.tile([128, 512], F32, tag="psA", name="psC")
                    for p in range(4):
                        nc.tensor.matmul(
                            ps[:],
                            wps[p][:],
                            yt_t[p][:, 512 * jq : 512 * (jq + 1)],
                            start=(p == 0),
                            stop=(p == 3),
                        )
                    ot = stagep.tile([128, 512], F32, tag="stage", name="stC")
                    nc.vector.tensor_copy(ot[:], ps[:])
                    nc.sync.dma_start(
                        outT[128 * e : 128 * (e + 1), 512 * jq : 512 * (jq + 1)],
                        ot[:],
                    )

    nc.compile()
    return nc


_CACHE: dict = {}
_LAST_IN_MAPS = None


def _get_nc(T: int):
    if T not in _CACHE:
        _CACHE[T] = build(T)
    return _CACHE[T]


def kernel(x: np.ndarray, w_attn: np.ndarray, w_proj: np.ndarray) -> np.ndarray:
    import ml_dtypes

    B, T, C_ = x.shape
    nc = _get_nc(T)
    bf = ml_dtypes.bfloat16
    kk = np.arange(128)[:, None]
    cc = np.arange(128)[None, :]
    tri = (cc >= kk).astype(bf)

    in_maps = []
    for core in range(8):
        b, g = core // 2, core % 2
        heads = range(8 * g, 8 * g + 8)
        rows = []
        for base in (0, C_, 2 * C_):  # q, k, v sections of w_attn
            for H in heads:
                rows.extend(range(base + 64 * H, base + 64 * H + 64))
        waT_l = np.ascontiguousarray(np.asarray(w_attn)[rows, :].T).astype(bf)
        dcols = [c for H in heads for c in range(64 * H, 64 * H + 64)]
        wpT_l = np.ascontiguousarray(np.asarray(w_proj)[:, dcols].T).astype(bf)
        xT_l = np.ascontiguousarray(np.asarray(x[b]).T).astype(bf)
        in_maps.append({"xT": xT_l, "waT": waT_l, "wpT": wpT_l, "tri": tri})

    global _LAST_IN_MAPS
    _LAST_IN_MAPS = in_maps
    res = bass_utils.run_bass_kernel_spmd(nc, in_maps, core_ids=list(range(8)))
    out = np.empty((B, T, C_), dtype=np.float32)
    for b in range(B):
        out[b] = (
            res.results[2 * b]["outT"].astype(np.float32)
            + res.results[2 * b + 1]["outT"].astype(np.float32)
        ).T
    return out
